# revision 65
# baseline (speedup 1.0000x reference)
"""Trainium2 Bass kernel for causal multi-head attention (dense transformer).

Problem (hardcoded): x [2, 2048, 1024], 16 heads x 64 dh, causal attention,
fp32 I/O. Sharding: 8 cores = 2 batches x 4 head-groups. Each core computes 4
heads for one batch plus a partial output projection [2048, 1024]; the host
sums the 4 partials per batch and adds b_O.

Everything on-device is computed in "transposed" orientation so no transposes
are needed anywhere:
  x^T (host-pretransposed)  ->  Q^T, K^T [dh, s] and V [s, dh] via matmuls
  S^T[k, q] = K Q^T         ->  P^T = exp(S^T / 8) (causal-masked)
  Z^T[dh, q] = V^T P^T      ->  normalized by column sums (ones-matmul)
  O^T[d, q] = W_O^T Z^T     (zt as the moving operand; host transposes)

Heads are processed in pairs: QK^T packs 2 heads in row-groups (0-63 / 64-127)
of the PE array, PV packs 2 heads in column-groups -- both run concurrently.

Schedule: attention q-blocks are software-pipelined (next scores emitted
before previous PV so the exp stream feeds the ACT engine early) and
interleaved with "filler" PE work (remaining QKV projection chains, then
O-projection chunks) so the PE stays busy while ACT churns exp. Input DMAs
are column-sliced and spread across idle engine queues. All on-device
compute is fp16; output partials are DMA'd as fp16 and summed on host.

Correctness note: this Tile build drops cross-engine waits for consumers of
DVE-written persist tiles (qt/kt/v/zt casts) -- first-run races showed up as
NaNs. Every such edge is therefore gated by a raw semaphore: the producing
DVE op is followed by a same-queue nop carrying then_inc, and consuming
matmuls/DMAs carry explicit wait_ops (see _sem_nop/_gate).
"""

import os
from contextlib import ExitStack

import numpy as np

import concourse.tile as tile
from concourse import bacc, mybir
from concourse.bass_utils import run_bass_kernel_spmd

# problem constants
B, S, DM, H, DH = 2, 2048, 1024, 16, 64
P = 128          # partitions
QB = 512         # q block (matmul moving free dim)
NKT = S // P     # 16 k tiles
NQB = S // QB    # 4 q blocks
NDM = DM // P    # 8 d_model tiles
HPC = 4          # heads per core
NCORES = 8

F32 = mybir.dt.float32
F16 = mybir.dt.float16
BF16 = mybir.dt.bfloat16

MM_DTYPE = os.environ.get("ATTN_MM_DTYPE", "fp16")

_PROGRAM_CACHE = {}
LAST_RESULTS = None  # BassKernelResults of the most recent run (for test.py)


def _mm(nc, out, lhsT, rhs, start, stop, skip=False):
    # skip_group_check: the sim's psum-group tracker doesn't distinguish
    # partition ranges; our concurrent groups in one bank are partition-disjoint
    # (rows 0-63 vs 64-127), which the per-partition zeroing model handles.
    return nc.tensor.matmul(
        out, lhsT, rhs, start=start, stop=stop, skip_group_check=skip
    )


def _chain(insts):
    """Same-engine ordering edges: pins the group's static queue order so a
    wait on the FIRST matmul gates the whole group (row/column-packed pairs
    still run concurrently on the array -- order only fixes dispatch)."""
    from concourse.tile import add_dep_helper

    for a, b in zip(insts[1:], insts):
        add_dep_helper(a.ins, b.ins, sync=True, reason="pack-pair order")


def _gate(nc, first_mm, waits, hint):
    """Gate a (chained) matmul group on raw semaphores. An instruction holds
    one raw wait; extra waits ride on tensor-queue nops ordered before the
    group's first matmul."""
    from concourse.tile import add_dep_helper

    sem, val = waits[0]
    first_mm.wait_op(sem, val, "sem-ge")
    for i, (sem, val) in enumerate(waits[1:]):
        tn = nc.tensor.nop(nofuse=True, hint=f"{hint}{i}")
        tn.wait_op(sem, val, "sem-ge")
        add_dep_helper(first_mm.ins, tn.ins, sync=True, reason="gate order")


def _sem_nop(nc, producers, sem, hint):
    """Vector-queue nop ordered after `producers` (same-engine sync edges)
    carrying a raw semaphore increment. This build's Tile scheduler elides
    cross-engine waits it believes are timing-covered -- unsoundly when the
    sim underestimates DVE/DMA latency -- so consumers of DVE-produced data
    wait on these raw semaphores instead."""
    from concourse.tile import add_dep_helper

    nop = nc.vector.nop(nofuse=True, hint=hint)
    for pr in producers:
        add_dep_helper(nop.ins, pr.ins, sync=True, reason="sem-nop order")
    nop.then_inc(sem)
    return nop


def build_program(mm_dtype=MM_DTYPE, with_bias=False):
    """Build the single-core SPMD Bass program (same program on all 8 cores)."""
    key = (mm_dtype, with_bias)
    if key in _PROGRAM_CACHE:
        return _PROGRAM_CACHE[key]

    MT = {"fp16": F16, "bf16": BF16, "fp32": F32}[mm_dtype]

    nc = bacc.Bacc(
        "TRN2", target_bir_lowering=False, debug=False, num_devices=NCORES
    )

    # ---- DRAM I/O (per-core shards, prearranged on host) ----
    xT_d = nc.dram_tensor("xT", [DM, S], MT, kind="ExternalInput")
    wqkv_d = nc.dram_tensor("wqkv", [DM, 3 * HPC * DH], MT, kind="ExternalInput")
    wo_d = nc.dram_tensor("wo", [P, 2 * DM], MT, kind="ExternalInput")
    bandm_d = nc.dram_tensor("bandm", [P, 2 * 2 * QB], MT, kind="ExternalInput")
    if with_bias:
        bq_d = nc.dram_tensor("bq", [2, P], F32, kind="ExternalInput")
        bk_d = nc.dram_tensor("bk", [2, P], F32, kind="ExternalInput")
        bv_d = nc.dram_tensor("bv", [P, HPC * DH], F32, kind="ExternalInput")
    # output is O^T [d_model, seq] so zt is the matmul's MOVING operand (its
    # cross-engine dependency is enforced; a zt lhsT read raced) -- the host
    # transposes while summing partials
    out_d = nc.dram_tensor("out", [DM, S], F16, kind="ExternalOutput")

    with tile.TileContext(nc) as tc, ExitStack() as ctx:
        const = ctx.enter_context(tc.tile_pool(name="const", bufs=1))
        persist = ctx.enter_context(tc.tile_pool(name="persist", bufs=1))

        # ---- persistent SBUF ----
        xt_sb = persist.tile([P, NDM, S], MT, name="xt_sb", tag="xt")
        w_sb = persist.tile([P, NDM, 3 * HPC * DH], MT, name="w_sb", tag="w")
        wo_sb = persist.tile([P, 2, DM], MT, name="wo_sb", tag="wo")
        bandm_sb = const.tile([P, 2, 2, QB], MT, name="bandm_sb", tag="bandm")
        ones64 = const.tile([P, 64], MT, name="ones64", tag="ones64")
        qt_sb = [persist.tile([P, S], MT, name=f"qt{p}", tag=f"qt{p}") for p in range(2)]
        kt_sb = [persist.tile([P, S], MT, name=f"kt{p}", tag=f"kt{p}") for p in range(2)]
        v_sb = [persist.tile([P, NKT, P], MT, name=f"v{p}", tag=f"v{p}") for p in range(2)]
        zt_sb = [persist.tile([P, S], MT, name=f"zt{p}", tag=f"zt{p}") for p in range(2)]

        if with_bias:
            bq_sb = const.tile([P, 2], F32, name="bq_sb", tag="bq")
            bk_sb = const.tile([P, 2], F32, name="bk_sb", tag="bk")
            bv_sb = const.tile([P, HPC * DH], F32, name="bv_sb", tag="bv")

        # ---- input DMAs: column-sliced x for an early start, spread across
        # otherwise-idle engine queues (issue cost ~0.6us per 128-row DMA) ----
        nc.gpsimd.memset(ones64[:], 1.0)
        # weight DMAs phased by projection (all wq slices, then wk, then wv)
        # so the Q chain unblocks ~2us earlier than with whole-row transfers
        for w0, w1 in ((0, 2 * P), (2 * P, 4 * P)):
            for t in range(NDM):
                eng = nc.sync if t % 2 == 0 else nc.scalar
                eng.dma_start(
                    out=w_sb[:, t, w0:w1], in_=wqkv_d[t * P : (t + 1) * P, w0:w1]
                )
                if w0 == 0:
                    nc.gpsimd.dma_start(
                        out=xt_sb[:, t, 0:QB], in_=xT_d[t * P : (t + 1) * P, 0:QB]
                    )
        for o in range(2):
            nc.sync.dma_start(
                out=bandm_sb[:, o, :, :],
                in_=bandm_d[:, o * 2 * QB : (o + 1) * 2 * QB],
            )
        for t in range(NDM):
            nc.scalar.dma_start(
                out=w_sb[:, t, 4 * P :], in_=wqkv_d[t * P : (t + 1) * P, 4 * P :]
            )
        for t in range(NDM):
            nc.scalar.dma_start(
                out=xt_sb[:, t, QB : 2 * QB],
                in_=xT_d[t * P : (t + 1) * P, QB : 2 * QB],
            )
        nc.scalar.dma_start(out=wo_sb[:, :, :], in_=wo_d[:, :])
        for t in range(NDM):
            nc.sync.dma_start(
                out=xt_sb[:, t, 2 * QB :], in_=xT_d[t * P : (t + 1) * P, 2 * QB :]
            )
        if with_bias:
            for p in range(2):
                nc.sync.dma_start(out=bq_sb[:, p : p + 1], in_=bq_d[p : p + 1, :])
                nc.sync.dma_start(out=bk_sb[:, p : p + 1], in_=bk_d[p : p + 1, :])
            nc.sync.dma_start(out=bv_sb[:], in_=bv_d[:, :])

        # ---- psum pools: sp shared by scores / projections / O-chunks ----
        sp = ctx.enter_context(tc.tile_pool(name="sp", bufs=3, space="PSUM"))
        zp = ctx.enter_context(tc.tile_pool(name="zp", bufs=1, space="PSUM"))
        dp = ctx.enter_context(tc.tile_pool(name="dp", bufs=1, space="PSUM"))
        ppool = ctx.enter_context(tc.tile_pool(name="ppool", bufs=12))
        bcpool = ctx.enter_context(tc.tile_pool(name="bcpool", bufs=2))
        ost = ctx.enter_context(tc.tile_pool(name="ost", bufs=4))

        # raw semaphores forcing every DVE -> PE/DMA cross-engine sync that
        # Tile's scheduler has been observed to drop (first-run NaNs/garbage
        # from matmuls reading casts or zt before the producing DVE op ran)
        zsem = {
            (p, qb): nc.alloc_semaphore(name=f"zsem{p}_{qb}")
            for p in range(2)
            for qb in range(NQB)
        }
        qksem = {
            (p, w, ch): nc.alloc_semaphore(name=f"qksem{p}{w}{ch}")
            for p in range(2)
            for w in ("q", "k")
            for ch in range(NQB)
        }
        vsem = {
            (st): nc.alloc_semaphore(name=f"vsem{st}")
            for st in range(0, NKT, 2)
        }
        msem = {
            (p, qb): nc.alloc_semaphore(name=f"msem{p}_{qb}")
            for p in range(2)
            for qb in range(NQB)
        }
        osem = {
            (t, qb): nc.alloc_semaphore(name=f"osem{t}_{qb}")
            for t in range(NDM)
            for qb in range(NQB)
        }

        def qk_chain(p, ch, which):
            """One Q^T or K^T projection chain: [dh-pair(128), 512 q cols]."""
            base, dst = (0, qt_sb) if which == "q" else (2 * P, kt_sb)
            bias = None
            if with_bias:
                bias = bq_sb if which == "q" else bk_sb
            qp = sp.tile([P, 2, QB], F32, name="qp", tag="s")
            for t in range(NDM):
                _mm(
                    nc,
                    qp[:, 0, :],
                    w_sb[:, t, base + p * P : base + (p + 1) * P],
                    xt_sb[:, t, ch * QB : (ch + 1) * QB],
                    start=(t == 0),
                    stop=(t == NDM - 1),
                )
            out = dst[p][:, ch * QB : (ch + 1) * QB]
            if with_bias:
                inst = nc.vector.tensor_scalar_add(
                    out, qp[:, 0, :], bias[:, p : p + 1]
                )
            else:
                inst = nc.vector.tensor_copy(out, qp[:, 0, :])
            _sem_nop(nc, [inst], qksem[(p, which, ch)], f"qk{p}{which}{ch}")

        def v_chain2(st):
            """V rows [st*128, (st+2)*128) for all 4 heads (two 8-MM chains
            into one psum tile, one strided cast per head pair)."""
            vp = sp.tile([P, 2, QB], F32, name="vp", tag="s")
            for half in range(2):
                for t in range(NDM):
                    _mm(
                        nc,
                        vp[:, half, 0 : HPC * DH],
                        xt_sb[:, t, (st + half) * P : (st + half + 1) * P],
                        w_sb[:, t, 4 * P : 4 * P + HPC * DH],
                        start=(t == 0),
                        stop=(t == NDM - 1),
                    )
            casts = []
            for p in range(2):
                if with_bias:
                    for half in range(2):
                        casts.append(nc.vector.tensor_add(
                            v_sb[p][:, st + half, :],
                            vp[:, half, p * P : (p + 1) * P],
                            bv_sb[:, p * P : (p + 1) * P],
                        ))
                else:
                    casts.append(nc.vector.tensor_copy(
                        v_sb[p][:, st : st + 2, :], vp[:, :, p * P : (p + 1) * P]
                    ))
            _sem_nop(nc, casts, vsem[st], f"v{st}")

        def o_unit(t, qb):
            """Partial O^T chunk [t*128,(t+1)*128) x [qb*512,(qb+1)*512):
            two accumulating MMs (one per head pair) with zt as the moving
            operand, one cast, one DMA."""
            ops = sp.tile([P, 2, QB], F32, name="ops", tag="s")
            mms = []
            for pp in range(2):
                mms.append(_mm(
                    nc,
                    ops[:, 0, :],
                    wo_sb[:, pp, t * P : (t + 1) * P],
                    zt_sb[pp][:, qb * QB : (qb + 1) * QB],
                    start=(pp == 0),
                    stop=(pp == 1),
                ))
                mms[pp].wait_op(zsem[(pp, qb)], 1, "sem-ge")
            _chain(mms)
            ot = ost.tile([P, QB], F16, name="ot", tag="ot")
            ocast = nc.vector.tensor_copy(ot[:, :], ops[:, 0, :])
            # per-chunk sem so the output DMA reads ot only after the cast;
            # alternate issue queues so the tail's issues don't serialize
            _sem_nop(nc, [ocast], osem[(t, qb)], f"o{t}_{qb}")
            deng = nc.gpsimd if (t + qb) % 2 == 0 else nc.sync
            deng.dma_start(
                out=out_d[t * P : (t + 1) * P, qb * QB : (qb + 1) * QB],
                in_=ot[:, :],
            ).wait_op(osem[(t, qb)], 1, "sem-ge")

        # ---- filler iterator: independent PE work interleaved into the
        # attention kg-step stream to keep the PE busy while ACT does exp ----
        fillers = [
            ("v", 0), ("v", 2), ("qk", 1, 0, "q"), ("qk", 1, 0, "k"),        # b1
            ("qk", 0, 1, "q"), ("qk", 0, 1, "k"), ("v", 4), ("v", 6),        # b2
            ("qk", 1, 1, "q"), ("qk", 1, 1, "k"),
            ("qk", 0, 2, "q"), ("qk", 0, 2, "k"),
            ("v", 8), ("v", 10), ("qk", 0, 3, "q"), ("qk", 0, 3, "k"),       # b3
            ("v", 12), ("v", 14), ("qk", 1, 3, "q"), ("qk", 1, 3, "k"),
            ("qk", 1, 2, "q"), ("qk", 1, 2, "k"), ("o", 0, 0), ("o", 1, 0),  # b4
        ] + [("o", t, 0) for t in range(2, NDM)] + [("o", 0, 1), ("o", 1, 1)] \
          + [("o", t, 1) for t in range(2, NDM)] \
          + [("o", t, 3) for t in range(NDM)] \
          + [("o", t, 2) for t in range(NDM)]
        fill_i = [0]

        def pop_filler(n):
            for _ in range(n):
                if fill_i[0] >= len(fillers):
                    return
                f = fillers[fill_i[0]]
                fill_i[0] += 1
                if f[0] == "qk":
                    qk_chain(f[1], f[2], f[3])
                elif f[0] == "v":
                    v_chain2(f[1])
                else:
                    o_unit(f[1], f[2])

        def attention_qb(p, qb, plan):
            """One attention q-block, software-pipelined: scores/exp of step
            s are emitted before PV of step s-1 so the PE prioritizes feeding
            the ACT engine; plan[s] fillers are popped after each PV."""
            q0 = qb * QB
            nk = (qb + 1) * (QB // P)  # k tiles in causal range
            nkg = nk // 2
            zps = zp.tile([P, QB], F32, name="zps", tag="z")
            dnb = dp.tile([P, QB], F32, name="dnb", tag="d")

            def pv_dnb(pA, pB, kg):
                # PV (column-packed heads) + softmax denominators: the
                # ones-matmul sums P over k AND broadcasts over the 64
                # rows of each head half, accumulated in PSUM; all read
                # only the valid q range of their k-tile
                group = []
                for j in range(2):
                    kt = kg * 2 + j
                    c0 = max(kt * P - q0, 0)
                    group += [
                        _mm(
                            nc, zps[0:64, c0:QB], v_sb[p][:, kt, 0:64],
                            pA[:, j, c0:QB],
                            start=(kt == 0), stop=(kt == nk - 1), skip=True,
                        ),
                        _mm(
                            nc, zps[64:P, c0:QB], v_sb[p][:, kt, 64:P],
                            pB[:, j, c0:QB],
                            start=(kt == 0), stop=(kt == nk - 1), skip=True,
                        ),
                        _mm(
                            nc, dnb[0:64, c0:QB], ones64[:], pA[:, j, c0:QB],
                            start=(kt == 0), stop=(kt == nk - 1), skip=True,
                        ),
                        _mm(
                            nc, dnb[64:P, c0:QB], ones64[:], pB[:, j, c0:QB],
                            start=(kt == 0), stop=(kt == nk - 1), skip=True,
                        ),
                    ]
                _chain(group)
                waits = [(vsem[kg * 2], 1)]
                if kg >= 2 * qb:
                    # this k-group's P tiles were rewritten by the band mask
                    waits.append((msem[(p, qb)], kg - 2 * qb + 1))
                _gate(nc, group[0], waits, f"gpv{p}{qb}{kg}")

            def scores_exp(kg):
                # offs[j]: first valid q column of k-tile kg*2+j
                offs = [kg * 2 * P + j * P - q0 for j in range(2)]
                band = offs[0] >= 0
                deep = band and offs[0] >= 2 * P  # o=1 band k-group
                sA = sp.tile([P, 2, QB], F32, name="sA", tag="s")
                sB = sp.tile([P, 2, QB], F32, name="sB", tag="s")
                # the deep band k-group only computes scores from the first
                # valid column of its j0 tile; j1's [offs0,offs1) range is
                # real-but-masked so the shared exp/mask APs below never
                # read uninitialized PSUM
                c0 = offs[0] if deep else 0
                group = [
                    _mm(
                        nc,
                        stile[:, j, c0:QB],
                        kt_sb[p][rows, (kg * 2 + j) * P : (kg * 2 + j + 1) * P],
                        qt_sb[p][rows, q0 + c0 : q0 + QB],
                        start=True,
                        stop=True,
                    )
                    for j in range(2)
                    for rows, stile in ((slice(0, 64), sA), (slice(64, P), sB))
                ]
                _chain(group)
                _gate(
                    nc,
                    group[0],
                    [(qksem[(p, "k", kg // 2)], 1), (qksem[(p, "q", qb)], 1)],
                    f"gsc{p}{qb}{kg}",
                )
                pA = ppool.tile([P, 2, QB], MT, name="pA", tag="pt")
                pB = ppool.tile([P, 2, QB], MT, name="pB", tag="pt")
                # exp(S/sqrt(dh)); scale folded into ACT
                if deep:
                    o0 = offs[0]
                    nc.scalar.activation(
                        pA[:, :, o0:], sA[:, :, o0:],
                        mybir.ActivationFunctionType.Exp, scale=0.125,
                    )
                    nc.scalar.activation(
                        pB[:, :, o0:], sB[:, :, o0:],
                        mybir.ActivationFunctionType.Exp, scale=0.125,
                    )
                    masks = [
                        nc.vector.tensor_mul(
                            pA[:, :, o0:], pA[:, :, o0:], bandm_sb[:, 1, :, o0:]
                        ),
                        nc.vector.tensor_mul(
                            pB[:, :, o0:], pB[:, :, o0:], bandm_sb[:, 1, :, o0:]
                        ),
                    ]
                    _sem_nop(nc, masks, msem[(p, qb)], f"m{p}{qb}d")
                else:
                    nc.scalar.activation(
                        pA[:], sA[:], mybir.ActivationFunctionType.Exp,
                        scale=0.125,
                    )
                    nc.scalar.activation(
                        pB[:], sB[:], mybir.ActivationFunctionType.Exp,
                        scale=0.125,
                    )
                    if band:
                        # causal mask: multiply diagonal-band tiles by 0/1
                        masks = [
                            nc.vector.tensor_mul(pA[:], pA[:], bandm_sb[:, 0, :, :]),
                            nc.vector.tensor_mul(pB[:], pB[:], bandm_sb[:, 0, :, :]),
                        ]
                        _sem_nop(nc, masks, msem[(p, qb)], f"m{p}{qb}b")
                return pA, pB

            pts = []
            for kg in range(nkg):
                pts.append(scores_exp(kg))
                if kg > 0:
                    pop_filler(plan[kg - 1])
                    pv_dnb(*pts[kg - 1], kg - 1)
            pop_filler(plan[nkg - 1])
            pv_dnb(*pts[nkg - 1], nkg - 1)

            bcs = bcpool.tile([P, QB], F32, name="bcs", tag="bcs")
            bcr = bcpool.tile([P, QB], F32, name="bcr", tag="bcr")
            nc.vector.reciprocal_approx_accurate(
                out=bcr[:], in_=dnb[:], scratch=bcs[:]
            )
            zi = nc.vector.tensor_mul(zt_sb[p][:, q0 : q0 + QB], zps[:], bcr[:])
            _sem_nop(nc, [zi], zsem[(p, qb)], f"zt{p}{qb}")

        # ---- emission: minimal prelude (first Q/K chains), then the
        # attention blocks ordered so exp starts early and O-projection row
        # groups unlock in time to fill the late blocks ----
        qk_chain(0, 0, "q")
        qk_chain(0, 0, "k")
        attention_qb(0, 0, [2, 2])
        attention_qb(1, 0, [2, 2])
        attention_qb(0, 1, [2, 2, 2, 2])
        attention_qb(1, 1, [2, 2, 2, 2])
        attention_qb(0, 3, [1] * 8)
        attention_qb(1, 3, [1] * 6 + [0] * 2)
        attention_qb(0, 2, [1] * 6)
        attention_qb(1, 2, [1, 1] + [0] * 4)
        # remaining O units (last q block) as the tail
        pop_filler(len(fillers))

    nc.compile()
    _PROGRAM_CACHE[key] = nc
    return nc


def make_in_maps(
    normalized_resid_pre, W_Q, W_K, W_V, W_O, b_Q, b_K, b_V, b_O,
    mm_dtype=MM_DTYPE, with_bias=False,
):
    """Shard + prearrange the full inputs into per-core input maps."""
    np_mt = {"fp16": np.float16, "fp32": np.float32}.get(mm_dtype)
    if np_mt is None:
        import ml_dtypes  # noqa: F401  (registers bfloat16 with numpy)
        np_mt = np.dtype("bfloat16")

    x = np.asarray(normalized_resid_pre, dtype=np.float32)
    W_Q = np.asarray(W_Q, dtype=np.float32)
    W_K = np.asarray(W_K, dtype=np.float32)
    W_V = np.asarray(W_V, dtype=np.float32)
    W_O = np.asarray(W_O, dtype=np.float32)
    b_Q = np.asarray(b_Q, dtype=np.float32)
    b_K = np.asarray(b_K, dtype=np.float32)
    b_V = np.asarray(b_V, dtype=np.float32)

    xT = [np.ascontiguousarray(x[b].T).astype(np_mt) for b in range(B)]
    # multiplicative causal band masks at k-group granularity: variant o
    # covers the two k-tiles at q-block offsets (2o*128, (2o+1)*128)
    kp = np.arange(P)[:, None]
    qc = np.arange(QB)[None, :]
    bandm = np.concatenate(
        [
            np.concatenate(
                [
                    np.where(qc < (2 * o + j) * P + kp,
                             np.float32(0.0), np.float32(1.0))
                    for j in range(2)
                ],
                axis=1,
            )
            for o in range(2)
        ],
        axis=1,
    ).astype(np_mt)

    in_maps = []
    for c in range(NCORES):
        b = c // (NCORES // B)
        heads = [HPC * (c % (NCORES // B)) + i for i in range(HPC)]
        wq = np.concatenate([W_Q[h] for h in heads], axis=1)
        wk = np.concatenate([W_K[h] for h in heads], axis=1)
        wv = np.concatenate([W_V[h] for h in heads], axis=1)
        wqkv = np.concatenate([wq, wk, wv], axis=1).astype(np_mt)
        wo_full = np.concatenate([W_O[h] for h in heads], axis=0)  # [256, 1024]
        wo = np.concatenate([wo_full[0:P], wo_full[P:]], axis=1).astype(np_mt)
        m = {
            "xT": np.ascontiguousarray(xT[b]),
            "wqkv": wqkv,
            "wo": np.ascontiguousarray(wo),
            "bandm": np.ascontiguousarray(bandm),
        }
        if with_bias:
            m["bq"] = np.stack(
                [
                    np.concatenate([b_Q[heads[0]], b_Q[heads[1]]]),
                    np.concatenate([b_Q[heads[2]], b_Q[heads[3]]]),
                ]
            ).astype(np.float32)
            m["bk"] = np.stack(
                [
                    np.concatenate([b_K[heads[0]], b_K[heads[1]]]),
                    np.concatenate([b_K[heads[2]], b_K[heads[3]]]),
                ]
            ).astype(np.float32)
            m["bv"] = np.tile(
                np.concatenate([b_V[h] for h in heads])[None, :], (P, 1)
            ).astype(np.float32)
        in_maps.append(m)
    return in_maps


def kernel(normalized_resid_pre, W_Q, W_K, W_V, W_O, b_Q, b_K, b_V, b_O):
    global LAST_RESULTS
    with_bias = any(
        np.any(np.asarray(bx)) for bx in (b_Q, b_K, b_V)
    )
    nc = build_program(MM_DTYPE, with_bias)
    in_maps = make_in_maps(
        normalized_resid_pre, W_Q, W_K, W_V, W_O, b_Q, b_K, b_V, b_O,
        MM_DTYPE, with_bias,
    )
    trace = os.environ.get("ATTN_TRACE", "0") == "1"
    res = run_bass_kernel_spmd(nc, in_maps, list(range(NCORES)), trace=trace)
    LAST_RESULTS = res

    b_O = np.asarray(b_O, dtype=np.float32)
    # per-core partials are O^T [d_model, seq]; transpose while summing
    parts = [np.asarray(res.results[c]["out"], dtype=np.float32) for c in range(NCORES)]
    npc = NCORES // B  # cores per batch
    out = np.stack(
        [sum(parts[b * npc : (b + 1) * npc]).T + b_O for b in range(B)]
    )
    return out.astype(np.float32)


# revision 66
# speedup vs baseline: 1.0456x; 1.0456x over previous
"""Trainium2 Bass kernel for causal multi-head attention (dense transformer).

Problem (hardcoded): x [2, 2048, 1024], 16 heads x 64 dh, causal attention,
fp32 I/O. Sharding: 8 cores = 2 batches x 4 head-groups. Each core computes 4
heads for one batch plus a partial output projection [2048, 1024]; the host
sums the 4 partials per batch and adds b_O.

Everything on-device is computed in "transposed" orientation so no transposes
are needed anywhere:
  x^T (host-pretransposed)  ->  Q^T, K^T [dh, s] and V [s, dh] via matmuls
  S^T[k, q] = K Q^T         ->  P^T = exp(S^T / 8) (causal-masked)
  Z^T[dh, q] = V^T P^T      ->  normalized by column sums (ones-matmul)
  O^T[d, q] = W_O^T Z^T     (zt as the moving operand; host transposes)

Heads are processed in pairs: QK^T packs 2 heads in row-groups (0-63 / 64-127)
of the PE array, PV packs 2 heads in column-groups -- both run concurrently.

Schedule: attention q-blocks are software-pipelined (next scores emitted
before previous PV so the exp stream feeds the ACT engine early) and
interleaved with "filler" PE work (remaining QKV projection chains, then
O-projection chunks) so the PE stays busy while ACT churns exp. Input DMAs
are column-sliced and spread across idle engine queues. All on-device
compute is fp16; output partials are DMA'd as fp16 and summed on host.

Correctness note: this Tile build drops cross-engine waits for consumers of
DVE-written persist tiles (qt/kt/v/zt casts) -- first-run races showed up as
NaNs. Every such edge is therefore gated by a raw semaphore: the producing
DVE op is followed by a same-queue nop carrying then_inc, and consuming
matmuls/DMAs carry explicit wait_ops (see _sem_nop/_gate).
"""

import os
from contextlib import ExitStack

import numpy as np

import concourse.tile as tile
from concourse import bacc, mybir
from concourse.bass_utils import run_bass_kernel_spmd

# problem constants
B, S, DM, H, DH = 2, 2048, 1024, 16, 64
P = 128          # partitions
QB = 512         # q block (matmul moving free dim)
NKT = S // P     # 16 k tiles
NQB = S // QB    # 4 q blocks
NDM = DM // P    # 8 d_model tiles
HPC = 4          # heads per core
NCORES = 8

F32 = mybir.dt.float32
F16 = mybir.dt.float16
BF16 = mybir.dt.bfloat16

MM_DTYPE = os.environ.get("ATTN_MM_DTYPE", "fp16")

_PROGRAM_CACHE = {}
LAST_RESULTS = None  # BassKernelResults of the most recent run (for test.py)


def _mm(nc, out, lhsT, rhs, start, stop, skip=False):
    # skip_group_check: the sim's psum-group tracker doesn't distinguish
    # partition ranges; our concurrent groups in one bank are partition-disjoint
    # (rows 0-63 vs 64-127), which the per-partition zeroing model handles.
    return nc.tensor.matmul(
        out, lhsT, rhs, start=start, stop=stop, skip_group_check=skip
    )


def _chain(insts):
    """Same-engine ordering edges: pins the group's static queue order so a
    wait on the FIRST matmul gates the whole group (row/column-packed pairs
    still run concurrently on the array -- order only fixes dispatch)."""
    from concourse.tile import add_dep_helper

    for a, b in zip(insts[1:], insts):
        add_dep_helper(a.ins, b.ins, sync=True, reason="pack-pair order")


def _gate(nc, first_mm, waits, hint):
    """Gate a (chained) matmul group on raw semaphores. An instruction holds
    one raw wait; extra waits ride on tensor-queue nops ordered before the
    group's first matmul."""
    from concourse.tile import add_dep_helper

    sem, val = waits[0]
    first_mm.wait_op(sem, val, "sem-ge")
    for i, (sem, val) in enumerate(waits[1:]):
        tn = nc.tensor.nop(nofuse=True, hint=f"{hint}{i}")
        tn.wait_op(sem, val, "sem-ge")
        add_dep_helper(first_mm.ins, tn.ins, sync=True, reason="gate order")


def _sem_nop(nc, producers, sem, hint):
    """Vector-queue nop ordered after `producers` (same-engine sync edges)
    carrying a raw semaphore increment. This build's Tile scheduler elides
    cross-engine waits it believes are timing-covered -- unsoundly when the
    sim underestimates DVE/DMA latency -- so consumers of DVE-produced data
    wait on these raw semaphores instead."""
    from concourse.tile import add_dep_helper

    nop = nc.vector.nop(nofuse=True, hint=hint)
    for pr in producers:
        add_dep_helper(nop.ins, pr.ins, sync=True, reason="sem-nop order")
    nop.then_inc(sem)
    return nop


def build_program(mm_dtype=MM_DTYPE, with_bias=False):
    """Build the single-core SPMD Bass program (same program on all 8 cores)."""
    key = (mm_dtype, with_bias)
    if key in _PROGRAM_CACHE:
        return _PROGRAM_CACHE[key]

    MT = {"fp16": F16, "bf16": BF16, "fp32": F32}[mm_dtype]

    nc = bacc.Bacc(
        "TRN2", target_bir_lowering=False, debug=False, num_devices=NCORES
    )

    # ---- DRAM I/O (per-core shards, prearranged on host) ----
    xT_d = nc.dram_tensor("xT", [DM, S], MT, kind="ExternalInput")
    wqkv_d = nc.dram_tensor("wqkv", [DM, 3 * HPC * DH], MT, kind="ExternalInput")
    wo_d = nc.dram_tensor("wo", [P, 2 * DM], MT, kind="ExternalInput")
    bandm_d = nc.dram_tensor("bandm", [P, 2 * 2 * QB], MT, kind="ExternalInput")
    if with_bias:
        bq_d = nc.dram_tensor("bq", [2, P], F32, kind="ExternalInput")
        bk_d = nc.dram_tensor("bk", [2, P], F32, kind="ExternalInput")
        bv_d = nc.dram_tensor("bv", [P, HPC * DH], F32, kind="ExternalInput")
    # output is O^T [d_model, seq] so zt is the matmul's MOVING operand (its
    # cross-engine dependency is enforced; a zt lhsT read raced) -- the host
    # transposes while summing partials
    out_d = nc.dram_tensor("out", [DM, S], F16, kind="ExternalOutput")

    with tile.TileContext(nc) as tc, ExitStack() as ctx:
        const = ctx.enter_context(tc.tile_pool(name="const", bufs=1))
        persist = ctx.enter_context(tc.tile_pool(name="persist", bufs=1))

        # ---- persistent SBUF ----
        xt_sb = persist.tile([P, NDM, S], MT, name="xt_sb", tag="xt")
        w_sb = persist.tile([P, NDM, 3 * HPC * DH], MT, name="w_sb", tag="w")
        wo_sb = persist.tile([P, 2, DM], MT, name="wo_sb", tag="wo")
        bandm_sb = const.tile([P, 2, 2, QB], MT, name="bandm_sb", tag="bandm")
        ones64 = const.tile([P, 64], MT, name="ones64", tag="ones64")
        qt_sb = [persist.tile([P, S], MT, name=f"qt{p}", tag=f"qt{p}") for p in range(2)]
        kt_sb = [persist.tile([P, S], MT, name=f"kt{p}", tag=f"kt{p}") for p in range(2)]
        v_sb = [persist.tile([P, NKT, P], MT, name=f"v{p}", tag=f"v{p}") for p in range(2)]
        zt_sb = [persist.tile([P, S], MT, name=f"zt{p}", tag=f"zt{p}") for p in range(2)]

        if with_bias:
            bq_sb = const.tile([P, 2], F32, name="bq_sb", tag="bq")
            bk_sb = const.tile([P, 2], F32, name="bk_sb", tag="bk")
            bv_sb = const.tile([P, HPC * DH], F32, name="bv_sb", tag="bv")

        # ---- input DMAs: column-sliced x for an early start, spread across
        # otherwise-idle engine queues (issue cost ~0.6us per 128-row DMA) ----
        nc.gpsimd.memset(ones64[:], 1.0)
        for t in range(NDM):
            eng = nc.sync if t % 2 == 0 else nc.scalar
            eng.dma_start(out=w_sb[:, t, :], in_=wqkv_d[t * P : (t + 1) * P, :])
            nc.gpsimd.dma_start(
                out=xt_sb[:, t, 0:QB], in_=xT_d[t * P : (t + 1) * P, 0:QB]
            )
        for o in range(2):
            nc.sync.dma_start(
                out=bandm_sb[:, o, :, :],
                in_=bandm_d[:, o * 2 * QB : (o + 1) * 2 * QB],
            )
        for t in range(NDM):
            nc.scalar.dma_start(
                out=xt_sb[:, t, QB : 2 * QB],
                in_=xT_d[t * P : (t + 1) * P, QB : 2 * QB],
            )
        nc.scalar.dma_start(out=wo_sb[:, :, :], in_=wo_d[:, :])
        for t in range(NDM):
            nc.sync.dma_start(
                out=xt_sb[:, t, 2 * QB :], in_=xT_d[t * P : (t + 1) * P, 2 * QB :]
            )
        if with_bias:
            for p in range(2):
                nc.sync.dma_start(out=bq_sb[:, p : p + 1], in_=bq_d[p : p + 1, :])
                nc.sync.dma_start(out=bk_sb[:, p : p + 1], in_=bk_d[p : p + 1, :])
            nc.sync.dma_start(out=bv_sb[:], in_=bv_d[:, :])

        # ---- psum pools: sp shared by scores / projections / O-chunks ----
        sp = ctx.enter_context(tc.tile_pool(name="sp", bufs=3, space="PSUM"))
        zp = ctx.enter_context(tc.tile_pool(name="zp", bufs=1, space="PSUM"))
        dp = ctx.enter_context(tc.tile_pool(name="dp", bufs=1, space="PSUM"))
        ppool = ctx.enter_context(tc.tile_pool(name="ppool", bufs=12))
        bcpool = ctx.enter_context(tc.tile_pool(name="bcpool", bufs=2))
        ost = ctx.enter_context(tc.tile_pool(name="ost", bufs=4))

        # raw semaphores forcing every DVE -> PE/DMA cross-engine sync that
        # Tile's scheduler has been observed to drop (first-run NaNs/garbage
        # from matmuls reading casts or zt before the producing DVE op ran)
        zsem = {
            (p, qb): nc.alloc_semaphore(name=f"zsem{p}_{qb}")
            for p in range(2)
            for qb in range(NQB)
        }
        qksem = {
            (p, w, ch): nc.alloc_semaphore(name=f"qksem{p}{w}{ch}")
            for p in range(2)
            for w in ("q", "k")
            for ch in range(NQB)
        }
        vsem = {
            (st): nc.alloc_semaphore(name=f"vsem{st}")
            for st in range(0, NKT, 2)
        }
        msem = {
            (p, qb): nc.alloc_semaphore(name=f"msem{p}_{qb}")
            for p in range(2)
            for qb in range(NQB)
        }
        osem = {
            (t, qb): nc.alloc_semaphore(name=f"osem{t}_{qb}")
            for t in range(NDM)
            for qb in range(NQB)
        }

        def qk_chain(p, ch, which):
            """One Q^T or K^T projection chain: [dh-pair(128), 512 q cols]."""
            base, dst = (0, qt_sb) if which == "q" else (2 * P, kt_sb)
            bias = None
            if with_bias:
                bias = bq_sb if which == "q" else bk_sb
            qp = sp.tile([P, 2, QB], F32, name="qp", tag="s")
            for t in range(NDM):
                _mm(
                    nc,
                    qp[:, 0, :],
                    w_sb[:, t, base + p * P : base + (p + 1) * P],
                    xt_sb[:, t, ch * QB : (ch + 1) * QB],
                    start=(t == 0),
                    stop=(t == NDM - 1),
                )
            out = dst[p][:, ch * QB : (ch + 1) * QB]
            if with_bias:
                inst = nc.vector.tensor_scalar_add(
                    out, qp[:, 0, :], bias[:, p : p + 1]
                )
            else:
                inst = nc.vector.tensor_copy(out, qp[:, 0, :])
            _sem_nop(nc, [inst], qksem[(p, which, ch)], f"qk{p}{which}{ch}")

        def v_chain2(st):
            """V rows [st*128, (st+2)*128) for all 4 heads (two 8-MM chains
            into one psum tile, one strided cast per head pair)."""
            vp = sp.tile([P, 2, QB], F32, name="vp", tag="s")
            for half in range(2):
                for t in range(NDM):
                    _mm(
                        nc,
                        vp[:, half, 0 : HPC * DH],
                        xt_sb[:, t, (st + half) * P : (st + half + 1) * P],
                        w_sb[:, t, 4 * P : 4 * P + HPC * DH],
                        start=(t == 0),
                        stop=(t == NDM - 1),
                    )
            casts = []
            for p in range(2):
                if with_bias:
                    for half in range(2):
                        casts.append(nc.vector.tensor_add(
                            v_sb[p][:, st + half, :],
                            vp[:, half, p * P : (p + 1) * P],
                            bv_sb[:, p * P : (p + 1) * P],
                        ))
                else:
                    casts.append(nc.vector.tensor_copy(
                        v_sb[p][:, st : st + 2, :], vp[:, :, p * P : (p + 1) * P]
                    ))
            _sem_nop(nc, casts, vsem[st], f"v{st}")

        def o_unit(t, qb):
            """Partial O^T chunk [t*128,(t+1)*128) x [qb*512,(qb+1)*512):
            two accumulating MMs (one per head pair) with zt as the moving
            operand, one cast, one DMA."""
            ops = sp.tile([P, 2, QB], F32, name="ops", tag="s")
            mms = []
            for pp in range(2):
                mms.append(_mm(
                    nc,
                    ops[:, 0, :],
                    wo_sb[:, pp, t * P : (t + 1) * P],
                    zt_sb[pp][:, qb * QB : (qb + 1) * QB],
                    start=(pp == 0),
                    stop=(pp == 1),
                ))
                mms[pp].wait_op(zsem[(pp, qb)], 1, "sem-ge")
            _chain(mms)
            ot = ost.tile([P, QB], F16, name="ot", tag="ot")
            ocast = nc.vector.tensor_copy(ot[:, :], ops[:, 0, :])
            # per-chunk sem so the output DMA reads ot only after the cast;
            # alternate issue queues so the tail's issues don't serialize
            _sem_nop(nc, [ocast], osem[(t, qb)], f"o{t}_{qb}")
            deng = nc.gpsimd if (t + qb) % 2 == 0 else nc.sync
            deng.dma_start(
                out=out_d[t * P : (t + 1) * P, qb * QB : (qb + 1) * QB],
                in_=ot[:, :],
            ).wait_op(osem[(t, qb)], 1, "sem-ge")

        # ---- filler iterator: independent PE work interleaved into the
        # attention kg-step stream to keep the PE busy while ACT does exp ----
        fillers = [
            ("v", 0), ("v", 2), ("qk", 1, 0, "q"), ("qk", 1, 0, "k"),        # b1
            ("qk", 0, 1, "q"), ("qk", 0, 1, "k"), ("v", 4), ("v", 6),        # b2
            ("qk", 1, 1, "q"), ("qk", 1, 1, "k"),
            ("qk", 0, 2, "q"), ("qk", 0, 2, "k"),
            ("v", 8), ("v", 10), ("qk", 0, 3, "q"), ("qk", 0, 3, "k"),       # b3
            ("v", 12), ("v", 14), ("qk", 1, 3, "q"), ("qk", 1, 3, "k"),
            ("qk", 1, 2, "q"), ("qk", 1, 2, "k"), ("o", 0, 0), ("o", 1, 0),  # b4
        ] + [("o", t, 0) for t in range(2, NDM)] + [("o", 0, 1), ("o", 1, 1)] \
          + [("o", t, 1) for t in range(2, NDM)] \
          + [("o", t, 3) for t in range(NDM)] \
          + [("o", t, 2) for t in range(NDM)]
        fill_i = [0]

        def pop_filler(n):
            for _ in range(n):
                if fill_i[0] >= len(fillers):
                    return
                f = fillers[fill_i[0]]
                fill_i[0] += 1
                if f[0] == "qk":
                    qk_chain(f[1], f[2], f[3])
                elif f[0] == "v":
                    v_chain2(f[1])
                else:
                    o_unit(f[1], f[2])

        def attention_qb(p, qb, plan):
            """One attention q-block, software-pipelined: scores/exp of step
            s are emitted before PV of step s-1 so the PE prioritizes feeding
            the ACT engine; plan[s] fillers are popped after each PV."""
            q0 = qb * QB
            nk = (qb + 1) * (QB // P)  # k tiles in causal range
            nkg = nk // 2
            zps = zp.tile([P, QB], F32, name="zps", tag="z")
            dnb = dp.tile([P, QB], F32, name="dnb", tag="d")

            def pv_dnb(pA, pB, kg):
                # PV (column-packed heads) + softmax denominators: the
                # ones-matmul sums P over k AND broadcasts over the 64
                # rows of each head half, accumulated in PSUM; all read
                # only the valid q range of their k-tile
                group = []
                for j in range(2):
                    kt = kg * 2 + j
                    c0 = max(kt * P - q0, 0)
                    group += [
                        _mm(
                            nc, zps[0:64, c0:QB], v_sb[p][:, kt, 0:64],
                            pA[:, j, c0:QB],
                            start=(kt == 0), stop=(kt == nk - 1), skip=True,
                        ),
                        _mm(
                            nc, zps[64:P, c0:QB], v_sb[p][:, kt, 64:P],
                            pB[:, j, c0:QB],
                            start=(kt == 0), stop=(kt == nk - 1), skip=True,
                        ),
                        _mm(
                            nc, dnb[0:64, c0:QB], ones64[:], pA[:, j, c0:QB],
                            start=(kt == 0), stop=(kt == nk - 1), skip=True,
                        ),
                        _mm(
                            nc, dnb[64:P, c0:QB], ones64[:], pB[:, j, c0:QB],
                            start=(kt == 0), stop=(kt == nk - 1), skip=True,
                        ),
                    ]
                _chain(group)
                waits = [(vsem[kg * 2], 1)]
                if kg >= 2 * qb:
                    # this k-group's P tiles were rewritten by the band mask
                    waits.append((msem[(p, qb)], kg - 2 * qb + 1))
                _gate(nc, group[0], waits, f"gpv{p}{qb}{kg}")

            def scores_exp(kg):
                # offs[j]: first valid q column of k-tile kg*2+j
                offs = [kg * 2 * P + j * P - q0 for j in range(2)]
                band = offs[0] >= 0
                deep = band and offs[0] >= 2 * P  # o=1 band k-group
                sA = sp.tile([P, 2, QB], F32, name="sA", tag="s")
                sB = sp.tile([P, 2, QB], F32, name="sB", tag="s")
                # the deep band k-group only computes scores from the first
                # valid column of its j0 tile; j1's [offs0,offs1) range is
                # real-but-masked so the shared exp/mask APs below never
                # read uninitialized PSUM
                c0 = offs[0] if deep else 0
                group = [
                    _mm(
                        nc,
                        stile[:, j, c0:QB],
                        kt_sb[p][rows, (kg * 2 + j) * P : (kg * 2 + j + 1) * P],
                        qt_sb[p][rows, q0 + c0 : q0 + QB],
                        start=True,
                        stop=True,
                    )
                    for j in range(2)
                    for rows, stile in ((slice(0, 64), sA), (slice(64, P), sB))
                ]
                _chain(group)
                _gate(
                    nc,
                    group[0],
                    [(qksem[(p, "k", kg // 2)], 1), (qksem[(p, "q", qb)], 1)],
                    f"gsc{p}{qb}{kg}",
                )
                pA = ppool.tile([P, 2, QB], MT, name="pA", tag="pt")
                pB = ppool.tile([P, 2, QB], MT, name="pB", tag="pt")
                # exp(S/sqrt(dh)); scale folded into ACT
                if deep:
                    o0 = offs[0]
                    nc.scalar.activation(
                        pA[:, :, o0:], sA[:, :, o0:],
                        mybir.ActivationFunctionType.Exp, scale=0.125,
                    )
                    nc.scalar.activation(
                        pB[:, :, o0:], sB[:, :, o0:],
                        mybir.ActivationFunctionType.Exp, scale=0.125,
                    )
                    masks = [
                        nc.vector.tensor_mul(
                            pA[:, :, o0:], pA[:, :, o0:], bandm_sb[:, 1, :, o0:]
                        ),
                        nc.vector.tensor_mul(
                            pB[:, :, o0:], pB[:, :, o0:], bandm_sb[:, 1, :, o0:]
                        ),
                    ]
                    _sem_nop(nc, masks, msem[(p, qb)], f"m{p}{qb}d")
                else:
                    nc.scalar.activation(
                        pA[:], sA[:], mybir.ActivationFunctionType.Exp,
                        scale=0.125,
                    )
                    nc.scalar.activation(
                        pB[:], sB[:], mybir.ActivationFunctionType.Exp,
                        scale=0.125,
                    )
                    if band:
                        # causal mask: multiply diagonal-band tiles by 0/1
                        masks = [
                            nc.vector.tensor_mul(pA[:], pA[:], bandm_sb[:, 0, :, :]),
                            nc.vector.tensor_mul(pB[:], pB[:], bandm_sb[:, 0, :, :]),
                        ]
                        _sem_nop(nc, masks, msem[(p, qb)], f"m{p}{qb}b")
                return pA, pB

            pts = []
            for kg in range(nkg):
                pts.append(scores_exp(kg))
                if kg > 0:
                    pop_filler(plan[kg - 1])
                    pv_dnb(*pts[kg - 1], kg - 1)
            pop_filler(plan[nkg - 1])
            pv_dnb(*pts[nkg - 1], nkg - 1)

            bcs = bcpool.tile([P, QB], F32, name="bcs", tag="bcs")
            bcr = bcpool.tile([P, QB], F32, name="bcr", tag="bcr")
            nc.vector.reciprocal_approx_accurate(
                out=bcr[:], in_=dnb[:], scratch=bcs[:]
            )
            zi = nc.vector.tensor_mul(zt_sb[p][:, q0 : q0 + QB], zps[:], bcr[:])
            _sem_nop(nc, [zi], zsem[(p, qb)], f"zt{p}{qb}")

        # ---- emission: minimal prelude (first Q/K chains), then the
        # attention blocks ordered so exp starts early and O-projection row
        # groups unlock in time to fill the late blocks ----
        qk_chain(0, 0, "q")
        qk_chain(0, 0, "k")
        attention_qb(0, 0, [2, 2])
        attention_qb(1, 0, [2, 2])
        attention_qb(0, 1, [2, 2, 2, 2])
        attention_qb(1, 1, [2, 2, 2, 2])
        attention_qb(0, 3, [1] * 8)
        attention_qb(1, 3, [1] * 6 + [0] * 2)
        attention_qb(0, 2, [1] * 6)
        attention_qb(1, 2, [1, 1] + [0] * 4)
        # remaining O units (last q block) as the tail
        pop_filler(len(fillers))

    nc.compile()
    _PROGRAM_CACHE[key] = nc
    return nc


def make_in_maps(
    normalized_resid_pre, W_Q, W_K, W_V, W_O, b_Q, b_K, b_V, b_O,
    mm_dtype=MM_DTYPE, with_bias=False,
):
    """Shard + prearrange the full inputs into per-core input maps."""
    np_mt = {"fp16": np.float16, "fp32": np.float32}.get(mm_dtype)
    if np_mt is None:
        import ml_dtypes  # noqa: F401  (registers bfloat16 with numpy)
        np_mt = np.dtype("bfloat16")

    x = np.asarray(normalized_resid_pre, dtype=np.float32)
    W_Q = np.asarray(W_Q, dtype=np.float32)
    W_K = np.asarray(W_K, dtype=np.float32)
    W_V = np.asarray(W_V, dtype=np.float32)
    W_O = np.asarray(W_O, dtype=np.float32)
    b_Q = np.asarray(b_Q, dtype=np.float32)
    b_K = np.asarray(b_K, dtype=np.float32)
    b_V = np.asarray(b_V, dtype=np.float32)

    xT = [np.ascontiguousarray(x[b].T).astype(np_mt) for b in range(B)]
    # multiplicative causal band masks at k-group granularity: variant o
    # covers the two k-tiles at q-block offsets (2o*128, (2o+1)*128)
    kp = np.arange(P)[:, None]
    qc = np.arange(QB)[None, :]
    bandm = np.concatenate(
        [
            np.concatenate(
                [
                    np.where(qc < (2 * o + j) * P + kp,
                             np.float32(0.0), np.float32(1.0))
                    for j in range(2)
                ],
                axis=1,
            )
            for o in range(2)
        ],
        axis=1,
    ).astype(np_mt)

    in_maps = []
    for c in range(NCORES):
        b = c // (NCORES // B)
        heads = [HPC * (c % (NCORES // B)) + i for i in range(HPC)]
        wq = np.concatenate([W_Q[h] for h in heads], axis=1)
        wk = np.concatenate([W_K[h] for h in heads], axis=1)
        wv = np.concatenate([W_V[h] for h in heads], axis=1)
        wqkv = np.concatenate([wq, wk, wv], axis=1).astype(np_mt)
        wo_full = np.concatenate([W_O[h] for h in heads], axis=0)  # [256, 1024]
        wo = np.concatenate([wo_full[0:P], wo_full[P:]], axis=1).astype(np_mt)
        m = {
            "xT": np.ascontiguousarray(xT[b]),
            "wqkv": wqkv,
            "wo": np.ascontiguousarray(wo),
            "bandm": np.ascontiguousarray(bandm),
        }
        if with_bias:
            m["bq"] = np.stack(
                [
                    np.concatenate([b_Q[heads[0]], b_Q[heads[1]]]),
                    np.concatenate([b_Q[heads[2]], b_Q[heads[3]]]),
                ]
            ).astype(np.float32)
            m["bk"] = np.stack(
                [
                    np.concatenate([b_K[heads[0]], b_K[heads[1]]]),
                    np.concatenate([b_K[heads[2]], b_K[heads[3]]]),
                ]
            ).astype(np.float32)
            m["bv"] = np.tile(
                np.concatenate([b_V[h] for h in heads])[None, :], (P, 1)
            ).astype(np.float32)
        in_maps.append(m)
    return in_maps


def kernel(normalized_resid_pre, W_Q, W_K, W_V, W_O, b_Q, b_K, b_V, b_O):
    global LAST_RESULTS
    with_bias = any(
        np.any(np.asarray(bx)) for bx in (b_Q, b_K, b_V)
    )
    nc = build_program(MM_DTYPE, with_bias)
    in_maps = make_in_maps(
        normalized_resid_pre, W_Q, W_K, W_V, W_O, b_Q, b_K, b_V, b_O,
        MM_DTYPE, with_bias,
    )
    trace = os.environ.get("ATTN_TRACE", "0") == "1"
    res = run_bass_kernel_spmd(nc, in_maps, list(range(NCORES)), trace=trace)
    LAST_RESULTS = res

    b_O = np.asarray(b_O, dtype=np.float32)
    # per-core partials are O^T [d_model, seq]; transpose while summing
    parts = [np.asarray(res.results[c]["out"], dtype=np.float32) for c in range(NCORES)]
    npc = NCORES // B  # cores per batch
    out = np.stack(
        [sum(parts[b * npc : (b + 1) * npc]).T + b_O for b in range(B)]
    )
    return out.astype(np.float32)


# revision 67
# speedup vs baseline: 1.0461x; 1.0005x over previous
"""Trainium2 Bass kernel for causal multi-head attention (dense transformer).

Problem (hardcoded): x [2, 2048, 1024], 16 heads x 64 dh, causal attention,
fp32 I/O. Sharding: 8 cores = 2 batches x 4 head-groups. Each core computes 4
heads for one batch plus a partial output projection [2048, 1024]; the host
sums the 4 partials per batch and adds b_O.

Everything on-device is computed in "transposed" orientation so no transposes
are needed anywhere:
  x^T (host-pretransposed)  ->  Q^T, K^T [dh, s] and V [s, dh] via matmuls
  S^T[k, q] = K Q^T         ->  P^T = exp(S^T / 8) (causal-masked)
  Z^T[dh, q] = V^T P^T      ->  normalized by column sums (ones-matmul)
  O^T[d, q] = W_O^T Z^T     (zt as the moving operand; host transposes)

Heads are processed in pairs: QK^T packs 2 heads in row-groups (0-63 / 64-127)
of the PE array, PV packs 2 heads in column-groups -- both run concurrently.

Schedule: attention q-blocks are software-pipelined (next scores emitted
before previous PV so the exp stream feeds the ACT engine early) and
interleaved with "filler" PE work (remaining QKV projection chains, then
O-projection chunks) so the PE stays busy while ACT churns exp. Input DMAs
are column-sliced and spread across idle engine queues. All on-device
compute is fp16; output partials are DMA'd as fp16 and summed on host.

Correctness note: this Tile build drops cross-engine waits for consumers of
DVE-written persist tiles (qt/kt/v/zt casts) -- first-run races showed up as
NaNs. Every such edge is therefore gated by a raw semaphore: the producing
DVE op is followed by a same-queue nop carrying then_inc, and consuming
matmuls/DMAs carry explicit wait_ops (see _sem_nop/_gate).
"""

import os
from contextlib import ExitStack

import numpy as np

import concourse.tile as tile
from concourse import bacc, mybir
from concourse.bass_utils import run_bass_kernel_spmd

# problem constants
B, S, DM, H, DH = 2, 2048, 1024, 16, 64
P = 128          # partitions
QB = 512         # q block (matmul moving free dim)
NKT = S // P     # 16 k tiles
NQB = S // QB    # 4 q blocks
NDM = DM // P    # 8 d_model tiles
HPC = 4          # heads per core
NCORES = 8

F32 = mybir.dt.float32
F16 = mybir.dt.float16
BF16 = mybir.dt.bfloat16

MM_DTYPE = os.environ.get("ATTN_MM_DTYPE", "fp16")

_PROGRAM_CACHE = {}
LAST_RESULTS = None  # BassKernelResults of the most recent run (for test.py)


def _mm(nc, out, lhsT, rhs, start, stop, skip=False):
    # skip_group_check: the sim's psum-group tracker doesn't distinguish
    # partition ranges; our concurrent groups in one bank are partition-disjoint
    # (rows 0-63 vs 64-127), which the per-partition zeroing model handles.
    return nc.tensor.matmul(
        out, lhsT, rhs, start=start, stop=stop, skip_group_check=skip
    )


def _chain(insts):
    """Same-engine ordering edges: pins the group's static queue order so a
    wait on the FIRST matmul gates the whole group (row/column-packed pairs
    still run concurrently on the array -- order only fixes dispatch)."""
    from concourse.tile import add_dep_helper

    for a, b in zip(insts[1:], insts):
        add_dep_helper(a.ins, b.ins, sync=True, reason="pack-pair order")


def _gate(nc, first_mm, waits, hint):
    """Gate a (chained) matmul group on raw semaphores. An instruction holds
    one raw wait; extra waits ride on tensor-queue nops ordered before the
    group's first matmul."""
    from concourse.tile import add_dep_helper

    sem, val = waits[0]
    first_mm.wait_op(sem, val, "sem-ge")
    for i, (sem, val) in enumerate(waits[1:]):
        tn = nc.tensor.nop(nofuse=True, hint=f"{hint}{i}")
        tn.wait_op(sem, val, "sem-ge")
        add_dep_helper(first_mm.ins, tn.ins, sync=True, reason="gate order")


def _sem_nop(nc, producers, sem, hint):
    """Vector-queue nop ordered after `producers` (same-engine sync edges)
    carrying a raw semaphore increment. This build's Tile scheduler elides
    cross-engine waits it believes are timing-covered -- unsoundly when the
    sim underestimates DVE/DMA latency -- so consumers of DVE-produced data
    wait on these raw semaphores instead."""
    from concourse.tile import add_dep_helper

    nop = nc.vector.nop(nofuse=True, hint=hint)
    for pr in producers:
        add_dep_helper(nop.ins, pr.ins, sync=True, reason="sem-nop order")
    nop.then_inc(sem)
    return nop


def build_program(mm_dtype=MM_DTYPE, with_bias=False):
    """Build the single-core SPMD Bass program (same program on all 8 cores)."""
    key = (mm_dtype, with_bias)
    if key in _PROGRAM_CACHE:
        return _PROGRAM_CACHE[key]

    MT = {"fp16": F16, "bf16": BF16, "fp32": F32}[mm_dtype]

    nc = bacc.Bacc(
        "TRN2", target_bir_lowering=False, debug=False, num_devices=NCORES
    )

    # ---- DRAM I/O (per-core shards, prearranged on host) ----
    xT_d = nc.dram_tensor("xT", [DM, S], MT, kind="ExternalInput")
    wqkv_d = nc.dram_tensor("wqkv", [DM, 3 * HPC * DH], MT, kind="ExternalInput")
    wo_d = nc.dram_tensor("wo", [P, 2 * DM], MT, kind="ExternalInput")
    bandm_d = nc.dram_tensor("bandm", [P, 2 * 2 * QB], MT, kind="ExternalInput")
    if with_bias:
        bq_d = nc.dram_tensor("bq", [2, P], F32, kind="ExternalInput")
        bk_d = nc.dram_tensor("bk", [2, P], F32, kind="ExternalInput")
        bv_d = nc.dram_tensor("bv", [P, HPC * DH], F32, kind="ExternalInput")
    # output is O^T [d_model, seq] so zt is the matmul's MOVING operand (its
    # cross-engine dependency is enforced; a zt lhsT read raced) -- the host
    # transposes while summing partials
    out_d = nc.dram_tensor("out", [DM, S], F16, kind="ExternalOutput")

    with tile.TileContext(nc) as tc, ExitStack() as ctx:
        const = ctx.enter_context(tc.tile_pool(name="const", bufs=1))
        persist = ctx.enter_context(tc.tile_pool(name="persist", bufs=1))

        # ---- persistent SBUF ----
        xt_sb = persist.tile([P, NDM, S], MT, name="xt_sb", tag="xt")
        w_sb = persist.tile([P, NDM, 3 * HPC * DH], MT, name="w_sb", tag="w")
        wo_sb = persist.tile([P, 2, DM], MT, name="wo_sb", tag="wo")
        bandm_sb = const.tile([P, 2, 2, QB], MT, name="bandm_sb", tag="bandm")
        ones64 = const.tile([P, 64], MT, name="ones64", tag="ones64")
        qt_sb = [persist.tile([P, S], MT, name=f"qt{p}", tag=f"qt{p}") for p in range(2)]
        kt_sb = [persist.tile([P, S], MT, name=f"kt{p}", tag=f"kt{p}") for p in range(2)]
        v_sb = [persist.tile([P, NKT, P], MT, name=f"v{p}", tag=f"v{p}") for p in range(2)]
        zt_sb = [persist.tile([P, S], MT, name=f"zt{p}", tag=f"zt{p}") for p in range(2)]

        if with_bias:
            bq_sb = const.tile([P, 2], F32, name="bq_sb", tag="bq")
            bk_sb = const.tile([P, 2], F32, name="bk_sb", tag="bk")
            bv_sb = const.tile([P, HPC * DH], F32, name="bv_sb", tag="bv")

        # ---- input DMAs: column-sliced x for an early start, spread across
        # otherwise-idle engine queues (issue cost ~0.6us per 128-row DMA) ----
        nc.gpsimd.memset(ones64[:], 1.0)
        for t in range(NDM):
            eng = nc.sync if t % 2 == 0 else nc.scalar
            eng.dma_start(out=w_sb[:, t, :], in_=wqkv_d[t * P : (t + 1) * P, :])
            nc.gpsimd.dma_start(
                out=xt_sb[:, t, 0:QB], in_=xT_d[t * P : (t + 1) * P, 0:QB]
            )
        for o in range(2):
            nc.sync.dma_start(
                out=bandm_sb[:, o, :, :],
                in_=bandm_d[:, o * 2 * QB : (o + 1) * 2 * QB],
            )
        for t in range(NDM):
            nc.scalar.dma_start(
                out=xt_sb[:, t, QB : 2 * QB],
                in_=xT_d[t * P : (t + 1) * P, QB : 2 * QB],
            )
        nc.scalar.dma_start(out=wo_sb[:, :, :], in_=wo_d[:, :])
        for t in range(NDM):
            nc.sync.dma_start(
                out=xt_sb[:, t, 2 * QB :], in_=xT_d[t * P : (t + 1) * P, 2 * QB :]
            )
        if with_bias:
            for p in range(2):
                nc.sync.dma_start(out=bq_sb[:, p : p + 1], in_=bq_d[p : p + 1, :])
                nc.sync.dma_start(out=bk_sb[:, p : p + 1], in_=bk_d[p : p + 1, :])
            nc.sync.dma_start(out=bv_sb[:], in_=bv_d[:, :])

        # ---- psum pools: sp shared by scores / projections / O-chunks ----
        sp = ctx.enter_context(tc.tile_pool(name="sp", bufs=3, space="PSUM"))
        zp = ctx.enter_context(tc.tile_pool(name="zp", bufs=1, space="PSUM"))
        dp = ctx.enter_context(tc.tile_pool(name="dp", bufs=1, space="PSUM"))
        ppool = ctx.enter_context(tc.tile_pool(name="ppool", bufs=16))
        bcpool = ctx.enter_context(tc.tile_pool(name="bcpool", bufs=2))
        ost = ctx.enter_context(tc.tile_pool(name="ost", bufs=4))

        # raw semaphores forcing every DVE -> PE/DMA cross-engine sync that
        # Tile's scheduler has been observed to drop (first-run NaNs/garbage
        # from matmuls reading casts or zt before the producing DVE op ran)
        zsem = {
            (p, qb): nc.alloc_semaphore(name=f"zsem{p}_{qb}")
            for p in range(2)
            for qb in range(NQB)
        }
        qksem = {
            (p, w, ch): nc.alloc_semaphore(name=f"qksem{p}{w}{ch}")
            for p in range(2)
            for w in ("q", "k")
            for ch in range(NQB)
        }
        vsem = {
            (st): nc.alloc_semaphore(name=f"vsem{st}")
            for st in range(0, NKT, 2)
        }
        msem = {
            (p, qb): nc.alloc_semaphore(name=f"msem{p}_{qb}")
            for p in range(2)
            for qb in range(NQB)
        }
        osem = {
            (t, qb): nc.alloc_semaphore(name=f"osem{t}_{qb}")
            for t in range(NDM)
            for qb in range(NQB)
        }

        def qk_chain(p, ch, which):
            """One Q^T or K^T projection chain: [dh-pair(128), 512 q cols]."""
            base, dst = (0, qt_sb) if which == "q" else (2 * P, kt_sb)
            bias = None
            if with_bias:
                bias = bq_sb if which == "q" else bk_sb
            qp = sp.tile([P, 2, QB], F32, name="qp", tag="s")
            for t in range(NDM):
                _mm(
                    nc,
                    qp[:, 0, :],
                    w_sb[:, t, base + p * P : base + (p + 1) * P],
                    xt_sb[:, t, ch * QB : (ch + 1) * QB],
                    start=(t == 0),
                    stop=(t == NDM - 1),
                )
            out = dst[p][:, ch * QB : (ch + 1) * QB]
            if with_bias:
                inst = nc.vector.tensor_scalar_add(
                    out, qp[:, 0, :], bias[:, p : p + 1]
                )
            else:
                inst = nc.vector.tensor_copy(out, qp[:, 0, :])
            _sem_nop(nc, [inst], qksem[(p, which, ch)], f"qk{p}{which}{ch}")

        def v_chain2(st):
            """V rows [st*128, (st+2)*128) for all 4 heads (two 8-MM chains
            into one psum tile, one strided cast per head pair)."""
            vp = sp.tile([P, 2, QB], F32, name="vp", tag="s")
            for half in range(2):
                for t in range(NDM):
                    _mm(
                        nc,
                        vp[:, half, 0 : HPC * DH],
                        xt_sb[:, t, (st + half) * P : (st + half + 1) * P],
                        w_sb[:, t, 4 * P : 4 * P + HPC * DH],
                        start=(t == 0),
                        stop=(t == NDM - 1),
                    )
            casts = []
            for p in range(2):
                if with_bias:
                    for half in range(2):
                        casts.append(nc.vector.tensor_add(
                            v_sb[p][:, st + half, :],
                            vp[:, half, p * P : (p + 1) * P],
                            bv_sb[:, p * P : (p + 1) * P],
                        ))
                else:
                    casts.append(nc.vector.tensor_copy(
                        v_sb[p][:, st : st + 2, :], vp[:, :, p * P : (p + 1) * P]
                    ))
            _sem_nop(nc, casts, vsem[st], f"v{st}")

        def o_unit(t, qb):
            """Partial O^T chunk [t*128,(t+1)*128) x [qb*512,(qb+1)*512):
            two accumulating MMs (one per head pair) with zt as the moving
            operand, one cast, one DMA."""
            ops = sp.tile([P, 2, QB], F32, name="ops", tag="s")
            mms = []
            for pp in range(2):
                mms.append(_mm(
                    nc,
                    ops[:, 0, :],
                    wo_sb[:, pp, t * P : (t + 1) * P],
                    zt_sb[pp][:, qb * QB : (qb + 1) * QB],
                    start=(pp == 0),
                    stop=(pp == 1),
                ))
                mms[pp].wait_op(zsem[(pp, qb)], 1, "sem-ge")
            _chain(mms)
            ot = ost.tile([P, QB], F16, name="ot", tag="ot")
            ocast = nc.vector.tensor_copy(ot[:, :], ops[:, 0, :])
            # per-chunk sem so the output DMA reads ot only after the cast;
            # alternate issue queues so the tail's issues don't serialize
            _sem_nop(nc, [ocast], osem[(t, qb)], f"o{t}_{qb}")
            deng = nc.gpsimd if (t + qb) % 2 == 0 else nc.sync
            deng.dma_start(
                out=out_d[t * P : (t + 1) * P, qb * QB : (qb + 1) * QB],
                in_=ot[:, :],
            ).wait_op(osem[(t, qb)], 1, "sem-ge")

        # ---- filler iterator: independent PE work interleaved into the
        # attention kg-step stream to keep the PE busy while ACT does exp ----
        fillers = [
            ("v", 0), ("v", 2), ("qk", 1, 0, "q"), ("qk", 1, 0, "k"),        # b1
            ("qk", 0, 1, "q"), ("qk", 0, 1, "k"), ("v", 4), ("v", 6),        # b2
            ("qk", 1, 1, "q"), ("qk", 1, 1, "k"),
            ("qk", 0, 2, "q"), ("qk", 0, 2, "k"),
            ("v", 8), ("v", 10), ("qk", 0, 3, "q"), ("qk", 0, 3, "k"),       # b3
            ("v", 12), ("v", 14), ("qk", 1, 3, "q"), ("qk", 1, 3, "k"),
            ("qk", 1, 2, "q"), ("qk", 1, 2, "k"), ("o", 0, 0), ("o", 1, 0),  # b4
        ] + [("o", t, 0) for t in range(2, NDM)] + [("o", 0, 1), ("o", 1, 1)] \
          + [("o", t, 1) for t in range(2, NDM)] \
          + [("o", t, 3) for t in range(NDM)] \
          + [("o", t, 2) for t in range(NDM)]
        fill_i = [0]

        def pop_filler(n):
            for _ in range(n):
                if fill_i[0] >= len(fillers):
                    return
                f = fillers[fill_i[0]]
                fill_i[0] += 1
                if f[0] == "qk":
                    qk_chain(f[1], f[2], f[3])
                elif f[0] == "v":
                    v_chain2(f[1])
                else:
                    o_unit(f[1], f[2])

        def attention_qb(p, qb, plan):
            """One attention q-block, software-pipelined: scores/exp of step
            s are emitted before PV of step s-1 so the PE prioritizes feeding
            the ACT engine; plan[s] fillers are popped after each PV."""
            q0 = qb * QB
            nk = (qb + 1) * (QB // P)  # k tiles in causal range
            nkg = nk // 2
            zps = zp.tile([P, QB], F32, name="zps", tag="z")
            dnb = dp.tile([P, QB], F32, name="dnb", tag="d")

            def pv_dnb(pA, pB, kg):
                # PV (column-packed heads) + softmax denominators: the
                # ones-matmul sums P over k AND broadcasts over the 64
                # rows of each head half, accumulated in PSUM; all read
                # only the valid q range of their k-tile
                group = []
                for j in range(2):
                    kt = kg * 2 + j
                    c0 = max(kt * P - q0, 0)
                    group += [
                        _mm(
                            nc, zps[0:64, c0:QB], v_sb[p][:, kt, 0:64],
                            pA[:, j, c0:QB],
                            start=(kt == 0), stop=(kt == nk - 1), skip=True,
                        ),
                        _mm(
                            nc, zps[64:P, c0:QB], v_sb[p][:, kt, 64:P],
                            pB[:, j, c0:QB],
                            start=(kt == 0), stop=(kt == nk - 1), skip=True,
                        ),
                        _mm(
                            nc, dnb[0:64, c0:QB], ones64[:], pA[:, j, c0:QB],
                            start=(kt == 0), stop=(kt == nk - 1), skip=True,
                        ),
                        _mm(
                            nc, dnb[64:P, c0:QB], ones64[:], pB[:, j, c0:QB],
                            start=(kt == 0), stop=(kt == nk - 1), skip=True,
                        ),
                    ]
                _chain(group)
                waits = [(vsem[kg * 2], 1)]
                if kg >= 2 * qb:
                    # this k-group's P tiles were rewritten by the band mask
                    waits.append((msem[(p, qb)], kg - 2 * qb + 1))
                _gate(nc, group[0], waits, f"gpv{p}{qb}{kg}")

            def scores_exp(kg):
                # offs[j]: first valid q column of k-tile kg*2+j
                offs = [kg * 2 * P + j * P - q0 for j in range(2)]
                band = offs[0] >= 0
                deep = band and offs[0] >= 2 * P  # o=1 band k-group
                sA = sp.tile([P, 2, QB], F32, name="sA", tag="s")
                sB = sp.tile([P, 2, QB], F32, name="sB", tag="s")
                # the deep band k-group only computes scores from the first
                # valid column of its j0 tile; j1's [offs0,offs1) range is
                # real-but-masked so the shared exp/mask APs below never
                # read uninitialized PSUM
                c0 = offs[0] if deep else 0
                group = [
                    _mm(
                        nc,
                        stile[:, j, c0:QB],
                        kt_sb[p][rows, (kg * 2 + j) * P : (kg * 2 + j + 1) * P],
                        qt_sb[p][rows, q0 + c0 : q0 + QB],
                        start=True,
                        stop=True,
                    )
                    for j in range(2)
                    for rows, stile in ((slice(0, 64), sA), (slice(64, P), sB))
                ]
                _chain(group)
                _gate(
                    nc,
                    group[0],
                    [(qksem[(p, "k", kg // 2)], 1), (qksem[(p, "q", qb)], 1)],
                    f"gsc{p}{qb}{kg}",
                )
                pA = ppool.tile([P, 2, QB], MT, name="pA", tag="pt")
                pB = ppool.tile([P, 2, QB], MT, name="pB", tag="pt")
                # exp(S/sqrt(dh)); scale folded into ACT
                if deep:
                    o0 = offs[0]
                    nc.scalar.activation(
                        pA[:, :, o0:], sA[:, :, o0:],
                        mybir.ActivationFunctionType.Exp, scale=0.125,
                    )
                    nc.scalar.activation(
                        pB[:, :, o0:], sB[:, :, o0:],
                        mybir.ActivationFunctionType.Exp, scale=0.125,
                    )
                    masks = [
                        nc.vector.tensor_mul(
                            pA[:, :, o0:], pA[:, :, o0:], bandm_sb[:, 1, :, o0:]
                        ),
                        nc.vector.tensor_mul(
                            pB[:, :, o0:], pB[:, :, o0:], bandm_sb[:, 1, :, o0:]
                        ),
                    ]
                    _sem_nop(nc, masks, msem[(p, qb)], f"m{p}{qb}d")
                else:
                    nc.scalar.activation(
                        pA[:], sA[:], mybir.ActivationFunctionType.Exp,
                        scale=0.125,
                    )
                    nc.scalar.activation(
                        pB[:], sB[:], mybir.ActivationFunctionType.Exp,
                        scale=0.125,
                    )
                    if band:
                        # causal mask: multiply diagonal-band tiles by 0/1
                        masks = [
                            nc.vector.tensor_mul(pA[:], pA[:], bandm_sb[:, 0, :, :]),
                            nc.vector.tensor_mul(pB[:], pB[:], bandm_sb[:, 0, :, :]),
                        ]
                        _sem_nop(nc, masks, msem[(p, qb)], f"m{p}{qb}b")
                return pA, pB

            pts = []
            for kg in range(nkg):
                pts.append(scores_exp(kg))
                if kg > 0:
                    pop_filler(plan[kg - 1])
                    pv_dnb(*pts[kg - 1], kg - 1)
            pop_filler(plan[nkg - 1])
            pv_dnb(*pts[nkg - 1], nkg - 1)

            bcs = bcpool.tile([P, QB], F32, name="bcs", tag="bcs")
            bcr = bcpool.tile([P, QB], F32, name="bcr", tag="bcr")
            nc.vector.reciprocal_approx_accurate(
                out=bcr[:], in_=dnb[:], scratch=bcs[:]
            )
            zi = nc.vector.tensor_mul(zt_sb[p][:, q0 : q0 + QB], zps[:], bcr[:])
            _sem_nop(nc, [zi], zsem[(p, qb)], f"zt{p}{qb}")

        # ---- emission: minimal prelude (first Q/K chains), then the
        # attention blocks ordered so exp starts early and O-projection row
        # groups unlock in time to fill the late blocks ----
        qk_chain(0, 0, "q")
        qk_chain(0, 0, "k")
        attention_qb(0, 0, [2, 2])
        attention_qb(1, 0, [2, 2])
        attention_qb(0, 1, [2, 2, 2, 2])
        attention_qb(1, 1, [2, 2, 2, 2])
        attention_qb(0, 3, [1] * 8)
        attention_qb(1, 3, [1] * 6 + [0] * 2)
        attention_qb(0, 2, [1] * 6)
        attention_qb(1, 2, [1, 1] + [0] * 4)
        # remaining O units (last q block) as the tail
        pop_filler(len(fillers))

    nc.compile()
    _PROGRAM_CACHE[key] = nc
    return nc


def make_in_maps(
    normalized_resid_pre, W_Q, W_K, W_V, W_O, b_Q, b_K, b_V, b_O,
    mm_dtype=MM_DTYPE, with_bias=False,
):
    """Shard + prearrange the full inputs into per-core input maps."""
    np_mt = {"fp16": np.float16, "fp32": np.float32}.get(mm_dtype)
    if np_mt is None:
        import ml_dtypes  # noqa: F401  (registers bfloat16 with numpy)
        np_mt = np.dtype("bfloat16")

    x = np.asarray(normalized_resid_pre, dtype=np.float32)
    W_Q = np.asarray(W_Q, dtype=np.float32)
    W_K = np.asarray(W_K, dtype=np.float32)
    W_V = np.asarray(W_V, dtype=np.float32)
    W_O = np.asarray(W_O, dtype=np.float32)
    b_Q = np.asarray(b_Q, dtype=np.float32)
    b_K = np.asarray(b_K, dtype=np.float32)
    b_V = np.asarray(b_V, dtype=np.float32)

    xT = [np.ascontiguousarray(x[b].T).astype(np_mt) for b in range(B)]
    # multiplicative causal band masks at k-group granularity: variant o
    # covers the two k-tiles at q-block offsets (2o*128, (2o+1)*128)
    kp = np.arange(P)[:, None]
    qc = np.arange(QB)[None, :]
    bandm = np.concatenate(
        [
            np.concatenate(
                [
                    np.where(qc < (2 * o + j) * P + kp,
                             np.float32(0.0), np.float32(1.0))
                    for j in range(2)
                ],
                axis=1,
            )
            for o in range(2)
        ],
        axis=1,
    ).astype(np_mt)

    in_maps = []
    for c in range(NCORES):
        b = c // (NCORES // B)
        heads = [HPC * (c % (NCORES // B)) + i for i in range(HPC)]
        wq = np.concatenate([W_Q[h] for h in heads], axis=1)
        wk = np.concatenate([W_K[h] for h in heads], axis=1)
        wv = np.concatenate([W_V[h] for h in heads], axis=1)
        wqkv = np.concatenate([wq, wk, wv], axis=1).astype(np_mt)
        wo_full = np.concatenate([W_O[h] for h in heads], axis=0)  # [256, 1024]
        wo = np.concatenate([wo_full[0:P], wo_full[P:]], axis=1).astype(np_mt)
        m = {
            "xT": np.ascontiguousarray(xT[b]),
            "wqkv": wqkv,
            "wo": np.ascontiguousarray(wo),
            "bandm": np.ascontiguousarray(bandm),
        }
        if with_bias:
            m["bq"] = np.stack(
                [
                    np.concatenate([b_Q[heads[0]], b_Q[heads[1]]]),
                    np.concatenate([b_Q[heads[2]], b_Q[heads[3]]]),
                ]
            ).astype(np.float32)
            m["bk"] = np.stack(
                [
                    np.concatenate([b_K[heads[0]], b_K[heads[1]]]),
                    np.concatenate([b_K[heads[2]], b_K[heads[3]]]),
                ]
            ).astype(np.float32)
            m["bv"] = np.tile(
                np.concatenate([b_V[h] for h in heads])[None, :], (P, 1)
            ).astype(np.float32)
        in_maps.append(m)
    return in_maps


def kernel(normalized_resid_pre, W_Q, W_K, W_V, W_O, b_Q, b_K, b_V, b_O):
    global LAST_RESULTS
    with_bias = any(
        np.any(np.asarray(bx)) for bx in (b_Q, b_K, b_V)
    )
    nc = build_program(MM_DTYPE, with_bias)
    in_maps = make_in_maps(
        normalized_resid_pre, W_Q, W_K, W_V, W_O, b_Q, b_K, b_V, b_O,
        MM_DTYPE, with_bias,
    )
    trace = os.environ.get("ATTN_TRACE", "0") == "1"
    res = run_bass_kernel_spmd(nc, in_maps, list(range(NCORES)), trace=trace)
    LAST_RESULTS = res

    b_O = np.asarray(b_O, dtype=np.float32)
    # per-core partials are O^T [d_model, seq]; transpose while summing
    parts = [np.asarray(res.results[c]["out"], dtype=np.float32) for c in range(NCORES)]
    npc = NCORES // B  # cores per batch
    out = np.stack(
        [sum(parts[b * npc : (b + 1) * npc]).T + b_O for b in range(B)]
    )
    return out.astype(np.float32)


# revision 68
# speedup vs baseline: 1.0467x; 1.0005x over previous
"""Trainium2 Bass kernel for causal multi-head attention (dense transformer).

Problem (hardcoded): x [2, 2048, 1024], 16 heads x 64 dh, causal attention,
fp32 I/O. Sharding: 8 cores = 2 batches x 4 head-groups. Each core computes 4
heads for one batch plus a partial output projection [2048, 1024]; the host
sums the 4 partials per batch and adds b_O.

Everything on-device is computed in "transposed" orientation so no transposes
are needed anywhere:
  x^T (host-pretransposed)  ->  Q^T, K^T [dh, s] and V [s, dh] via matmuls
  S^T[k, q] = K Q^T         ->  P^T = exp(S^T / 8) (causal-masked)
  Z^T[dh, q] = V^T P^T      ->  normalized by column sums (ones-matmul)
  O^T[d, q] = W_O^T Z^T     (zt as the moving operand; host transposes)

Heads are processed in pairs: QK^T packs 2 heads in row-groups (0-63 / 64-127)
of the PE array, PV packs 2 heads in column-groups -- both run concurrently.

Schedule: attention q-blocks are software-pipelined (next scores emitted
before previous PV so the exp stream feeds the ACT engine early) and
interleaved with "filler" PE work (remaining QKV projection chains, then
O-projection chunks) so the PE stays busy while ACT churns exp. Input DMAs
are column-sliced and spread across idle engine queues. All on-device
compute is fp16; output partials are DMA'd as fp16 and summed on host.

Correctness note: this Tile build drops cross-engine waits for consumers of
DVE-written persist tiles (qt/kt/v/zt casts) -- first-run races showed up as
NaNs. Every such edge is therefore gated by a raw semaphore: the producing
DVE op is followed by a same-queue nop carrying then_inc, and consuming
matmuls/DMAs carry explicit wait_ops (see _sem_nop/_gate).
"""

import os
from contextlib import ExitStack

import numpy as np

import concourse.tile as tile
from concourse import bacc, mybir
from concourse.bass_utils import run_bass_kernel_spmd

# problem constants
B, S, DM, H, DH = 2, 2048, 1024, 16, 64
P = 128          # partitions
QB = 512         # q block (matmul moving free dim)
NKT = S // P     # 16 k tiles
NQB = S // QB    # 4 q blocks
NDM = DM // P    # 8 d_model tiles
HPC = 4          # heads per core
NCORES = 8

F32 = mybir.dt.float32
F16 = mybir.dt.float16
BF16 = mybir.dt.bfloat16

MM_DTYPE = os.environ.get("ATTN_MM_DTYPE", "fp16")

_PROGRAM_CACHE = {}
LAST_RESULTS = None  # BassKernelResults of the most recent run (for test.py)


def _mm(nc, out, lhsT, rhs, start, stop, skip=False):
    # skip_group_check: the sim's psum-group tracker doesn't distinguish
    # partition ranges; our concurrent groups in one bank are partition-disjoint
    # (rows 0-63 vs 64-127), which the per-partition zeroing model handles.
    return nc.tensor.matmul(
        out, lhsT, rhs, start=start, stop=stop, skip_group_check=skip
    )


def _chain(insts):
    """Same-engine ordering edges: pins the group's static queue order so a
    wait on the FIRST matmul gates the whole group (row/column-packed pairs
    still run concurrently on the array -- order only fixes dispatch)."""
    from concourse.tile import add_dep_helper

    for a, b in zip(insts[1:], insts):
        add_dep_helper(a.ins, b.ins, sync=True, reason="pack-pair order")


def _gate(nc, first_mm, waits, hint):
    """Gate a (chained) matmul group on raw semaphores. An instruction holds
    one raw wait; extra waits ride on tensor-queue nops ordered before the
    group's first matmul."""
    from concourse.tile import add_dep_helper

    sem, val = waits[0]
    first_mm.wait_op(sem, val, "sem-ge")
    for i, (sem, val) in enumerate(waits[1:]):
        tn = nc.tensor.nop(nofuse=True, hint=f"{hint}{i}")
        tn.wait_op(sem, val, "sem-ge")
        add_dep_helper(first_mm.ins, tn.ins, sync=True, reason="gate order")


def _sem_nop(nc, producers, sem, hint):
    """Vector-queue nop ordered after `producers` (same-engine sync edges)
    carrying a raw semaphore increment. This build's Tile scheduler elides
    cross-engine waits it believes are timing-covered -- unsoundly when the
    sim underestimates DVE/DMA latency -- so consumers of DVE-produced data
    wait on these raw semaphores instead."""
    from concourse.tile import add_dep_helper

    nop = nc.vector.nop(nofuse=True, hint=hint)
    for pr in producers:
        add_dep_helper(nop.ins, pr.ins, sync=True, reason="sem-nop order")
    nop.then_inc(sem)
    return nop


def build_program(mm_dtype=MM_DTYPE, with_bias=False):
    """Build the single-core SPMD Bass program (same program on all 8 cores)."""
    key = (mm_dtype, with_bias)
    if key in _PROGRAM_CACHE:
        return _PROGRAM_CACHE[key]

    MT = {"fp16": F16, "bf16": BF16, "fp32": F32}[mm_dtype]

    nc = bacc.Bacc(
        "TRN2", target_bir_lowering=False, debug=False, num_devices=NCORES
    )

    # ---- DRAM I/O (per-core shards, prearranged on host) ----
    xT_d = nc.dram_tensor("xT", [DM, S], MT, kind="ExternalInput")
    wqkv_d = nc.dram_tensor("wqkv", [DM, 3 * HPC * DH], MT, kind="ExternalInput")
    wo_d = nc.dram_tensor("wo", [P, 2 * DM], MT, kind="ExternalInput")
    bandm_d = nc.dram_tensor("bandm", [P, 2 * 2 * QB], MT, kind="ExternalInput")
    if with_bias:
        bq_d = nc.dram_tensor("bq", [2, P], F32, kind="ExternalInput")
        bk_d = nc.dram_tensor("bk", [2, P], F32, kind="ExternalInput")
        bv_d = nc.dram_tensor("bv", [P, HPC * DH], F32, kind="ExternalInput")
    # output is O^T [d_model, seq] so zt is the matmul's MOVING operand (its
    # cross-engine dependency is enforced; a zt lhsT read raced) -- the host
    # transposes while summing partials
    out_d = nc.dram_tensor("out", [DM, S], F16, kind="ExternalOutput")

    with tile.TileContext(nc) as tc, ExitStack() as ctx:
        const = ctx.enter_context(tc.tile_pool(name="const", bufs=1))
        persist = ctx.enter_context(tc.tile_pool(name="persist", bufs=1))

        # ---- persistent SBUF ----
        xt_sb = persist.tile([P, NDM, S], MT, name="xt_sb", tag="xt")
        w_sb = persist.tile([P, NDM, 3 * HPC * DH], MT, name="w_sb", tag="w")
        wo_sb = persist.tile([P, 2, DM], MT, name="wo_sb", tag="wo")
        bandm_sb = const.tile([P, 2, 2, QB], MT, name="bandm_sb", tag="bandm")
        ones64 = const.tile([P, 64], MT, name="ones64", tag="ones64")
        qt_sb = [persist.tile([P, S], MT, name=f"qt{p}", tag=f"qt{p}") for p in range(2)]
        kt_sb = [persist.tile([P, S], MT, name=f"kt{p}", tag=f"kt{p}") for p in range(2)]
        v_sb = [persist.tile([P, NKT, P], MT, name=f"v{p}", tag=f"v{p}") for p in range(2)]
        zt_sb = [persist.tile([P, S], MT, name=f"zt{p}", tag=f"zt{p}") for p in range(2)]

        if with_bias:
            bq_sb = const.tile([P, 2], F32, name="bq_sb", tag="bq")
            bk_sb = const.tile([P, 2], F32, name="bk_sb", tag="bk")
            bv_sb = const.tile([P, HPC * DH], F32, name="bv_sb", tag="bv")

        # ---- input DMAs: column-sliced x for an early start, spread across
        # otherwise-idle engine queues (issue cost ~0.6us per 128-row DMA) ----
        nc.gpsimd.memset(ones64[:], 1.0)
        # first-needed tiles (weights + x columns 0:512) split across three
        # queues so no single stream paces the opening projection chains
        for t in range(NDM):
            weng = nc.sync if t % 2 == 0 else nc.scalar
            weng.dma_start(out=w_sb[:, t, :], in_=wqkv_d[t * P : (t + 1) * P, :])
            xeng = nc.gpsimd if t % 2 == 0 else nc.sync
            xeng.dma_start(
                out=xt_sb[:, t, 0:QB], in_=xT_d[t * P : (t + 1) * P, 0:QB]
            )
        for o in range(2):
            nc.sync.dma_start(
                out=bandm_sb[:, o, :, :],
                in_=bandm_d[:, o * 2 * QB : (o + 1) * 2 * QB],
            )
        for t in range(NDM):
            nc.scalar.dma_start(
                out=xt_sb[:, t, QB : 2 * QB],
                in_=xT_d[t * P : (t + 1) * P, QB : 2 * QB],
            )
        nc.scalar.dma_start(out=wo_sb[:, :, :], in_=wo_d[:, :])
        for t in range(NDM):
            nc.sync.dma_start(
                out=xt_sb[:, t, 2 * QB :], in_=xT_d[t * P : (t + 1) * P, 2 * QB :]
            )
        if with_bias:
            for p in range(2):
                nc.sync.dma_start(out=bq_sb[:, p : p + 1], in_=bq_d[p : p + 1, :])
                nc.sync.dma_start(out=bk_sb[:, p : p + 1], in_=bk_d[p : p + 1, :])
            nc.sync.dma_start(out=bv_sb[:], in_=bv_d[:, :])

        # ---- psum pools: sp shared by scores / projections / O-chunks ----
        sp = ctx.enter_context(tc.tile_pool(name="sp", bufs=3, space="PSUM"))
        zp = ctx.enter_context(tc.tile_pool(name="zp", bufs=1, space="PSUM"))
        dp = ctx.enter_context(tc.tile_pool(name="dp", bufs=1, space="PSUM"))
        ppool = ctx.enter_context(tc.tile_pool(name="ppool", bufs=16))
        bcpool = ctx.enter_context(tc.tile_pool(name="bcpool", bufs=2))
        ost = ctx.enter_context(tc.tile_pool(name="ost", bufs=4))

        # raw semaphores forcing every DVE -> PE/DMA cross-engine sync that
        # Tile's scheduler has been observed to drop (first-run NaNs/garbage
        # from matmuls reading casts or zt before the producing DVE op ran)
        zsem = {
            (p, qb): nc.alloc_semaphore(name=f"zsem{p}_{qb}")
            for p in range(2)
            for qb in range(NQB)
        }
        qksem = {
            (p, w, ch): nc.alloc_semaphore(name=f"qksem{p}{w}{ch}")
            for p in range(2)
            for w in ("q", "k")
            for ch in range(NQB)
        }
        vsem = {
            (st): nc.alloc_semaphore(name=f"vsem{st}")
            for st in range(0, NKT, 2)
        }
        msem = {
            (p, qb): nc.alloc_semaphore(name=f"msem{p}_{qb}")
            for p in range(2)
            for qb in range(NQB)
        }
        osem = {
            (t, qb): nc.alloc_semaphore(name=f"osem{t}_{qb}")
            for t in range(NDM)
            for qb in range(NQB)
        }

        def qk_chain(p, ch, which):
            """One Q^T or K^T projection chain: [dh-pair(128), 512 q cols]."""
            base, dst = (0, qt_sb) if which == "q" else (2 * P, kt_sb)
            bias = None
            if with_bias:
                bias = bq_sb if which == "q" else bk_sb
            qp = sp.tile([P, 2, QB], F32, name="qp", tag="s")
            for t in range(NDM):
                _mm(
                    nc,
                    qp[:, 0, :],
                    w_sb[:, t, base + p * P : base + (p + 1) * P],
                    xt_sb[:, t, ch * QB : (ch + 1) * QB],
                    start=(t == 0),
                    stop=(t == NDM - 1),
                )
            out = dst[p][:, ch * QB : (ch + 1) * QB]
            if with_bias:
                inst = nc.vector.tensor_scalar_add(
                    out, qp[:, 0, :], bias[:, p : p + 1]
                )
            else:
                inst = nc.vector.tensor_copy(out, qp[:, 0, :])
            _sem_nop(nc, [inst], qksem[(p, which, ch)], f"qk{p}{which}{ch}")

        def v_chain2(st):
            """V rows [st*128, (st+2)*128) for all 4 heads (two 8-MM chains
            into one psum tile, one strided cast per head pair)."""
            vp = sp.tile([P, 2, QB], F32, name="vp", tag="s")
            for half in range(2):
                for t in range(NDM):
                    _mm(
                        nc,
                        vp[:, half, 0 : HPC * DH],
                        xt_sb[:, t, (st + half) * P : (st + half + 1) * P],
                        w_sb[:, t, 4 * P : 4 * P + HPC * DH],
                        start=(t == 0),
                        stop=(t == NDM - 1),
                    )
            casts = []
            for p in range(2):
                if with_bias:
                    for half in range(2):
                        casts.append(nc.vector.tensor_add(
                            v_sb[p][:, st + half, :],
                            vp[:, half, p * P : (p + 1) * P],
                            bv_sb[:, p * P : (p + 1) * P],
                        ))
                else:
                    casts.append(nc.vector.tensor_copy(
                        v_sb[p][:, st : st + 2, :], vp[:, :, p * P : (p + 1) * P]
                    ))
            _sem_nop(nc, casts, vsem[st], f"v{st}")

        def o_unit(t, qb):
            """Partial O^T chunk [t*128,(t+1)*128) x [qb*512,(qb+1)*512):
            two accumulating MMs (one per head pair) with zt as the moving
            operand, one cast, one DMA."""
            ops = sp.tile([P, 2, QB], F32, name="ops", tag="s")
            mms = []
            for pp in range(2):
                mms.append(_mm(
                    nc,
                    ops[:, 0, :],
                    wo_sb[:, pp, t * P : (t + 1) * P],
                    zt_sb[pp][:, qb * QB : (qb + 1) * QB],
                    start=(pp == 0),
                    stop=(pp == 1),
                ))
                mms[pp].wait_op(zsem[(pp, qb)], 1, "sem-ge")
            _chain(mms)
            ot = ost.tile([P, QB], F16, name="ot", tag="ot")
            ocast = nc.vector.tensor_copy(ot[:, :], ops[:, 0, :])
            # per-chunk sem so the output DMA reads ot only after the cast;
            # alternate issue queues so the tail's issues don't serialize
            _sem_nop(nc, [ocast], osem[(t, qb)], f"o{t}_{qb}")
            deng = nc.gpsimd if (t + qb) % 2 == 0 else nc.sync
            deng.dma_start(
                out=out_d[t * P : (t + 1) * P, qb * QB : (qb + 1) * QB],
                in_=ot[:, :],
            ).wait_op(osem[(t, qb)], 1, "sem-ge")

        # ---- filler iterator: independent PE work interleaved into the
        # attention kg-step stream to keep the PE busy while ACT does exp ----
        fillers = [
            ("v", 0), ("v", 2), ("qk", 1, 0, "q"), ("qk", 1, 0, "k"),        # b1
            ("qk", 0, 1, "q"), ("qk", 0, 1, "k"), ("v", 4), ("v", 6),        # b2
            ("qk", 1, 1, "q"), ("qk", 1, 1, "k"),
            ("qk", 0, 2, "q"), ("qk", 0, 2, "k"),
            ("v", 8), ("v", 10), ("qk", 0, 3, "q"), ("qk", 0, 3, "k"),       # b3
            ("v", 12), ("v", 14), ("qk", 1, 3, "q"), ("qk", 1, 3, "k"),
            ("qk", 1, 2, "q"), ("qk", 1, 2, "k"), ("o", 0, 0), ("o", 1, 0),  # b4
        ] + [("o", t, 0) for t in range(2, NDM)] + [("o", 0, 1), ("o", 1, 1)] \
          + [("o", t, 1) for t in range(2, NDM)] \
          + [("o", t, 3) for t in range(NDM)] \
          + [("o", t, 2) for t in range(NDM)]
        fill_i = [0]

        def pop_filler(n):
            for _ in range(n):
                if fill_i[0] >= len(fillers):
                    return
                f = fillers[fill_i[0]]
                fill_i[0] += 1
                if f[0] == "qk":
                    qk_chain(f[1], f[2], f[3])
                elif f[0] == "v":
                    v_chain2(f[1])
                else:
                    o_unit(f[1], f[2])

        def attention_qb(p, qb, plan):
            """One attention q-block, software-pipelined: scores/exp of step
            s are emitted before PV of step s-1 so the PE prioritizes feeding
            the ACT engine; plan[s] fillers are popped after each PV."""
            q0 = qb * QB
            nk = (qb + 1) * (QB // P)  # k tiles in causal range
            nkg = nk // 2
            zps = zp.tile([P, QB], F32, name="zps", tag="z")
            dnb = dp.tile([P, QB], F32, name="dnb", tag="d")

            def pv_dnb(pA, pB, kg):
                # PV (column-packed heads) + softmax denominators: the
                # ones-matmul sums P over k AND broadcasts over the 64
                # rows of each head half, accumulated in PSUM; all read
                # only the valid q range of their k-tile
                group = []
                for j in range(2):
                    kt = kg * 2 + j
                    c0 = max(kt * P - q0, 0)
                    group += [
                        _mm(
                            nc, zps[0:64, c0:QB], v_sb[p][:, kt, 0:64],
                            pA[:, j, c0:QB],
                            start=(kt == 0), stop=(kt == nk - 1), skip=True,
                        ),
                        _mm(
                            nc, zps[64:P, c0:QB], v_sb[p][:, kt, 64:P],
                            pB[:, j, c0:QB],
                            start=(kt == 0), stop=(kt == nk - 1), skip=True,
                        ),
                        _mm(
                            nc, dnb[0:64, c0:QB], ones64[:], pA[:, j, c0:QB],
                            start=(kt == 0), stop=(kt == nk - 1), skip=True,
                        ),
                        _mm(
                            nc, dnb[64:P, c0:QB], ones64[:], pB[:, j, c0:QB],
                            start=(kt == 0), stop=(kt == nk - 1), skip=True,
                        ),
                    ]
                _chain(group)
                waits = [(vsem[kg * 2], 1)]
                if kg >= 2 * qb:
                    # this k-group's P tiles were rewritten by the band mask
                    waits.append((msem[(p, qb)], kg - 2 * qb + 1))
                _gate(nc, group[0], waits, f"gpv{p}{qb}{kg}")

            def scores_exp(kg):
                # offs[j]: first valid q column of k-tile kg*2+j
                offs = [kg * 2 * P + j * P - q0 for j in range(2)]
                band = offs[0] >= 0
                deep = band and offs[0] >= 2 * P  # o=1 band k-group
                sA = sp.tile([P, 2, QB], F32, name="sA", tag="s")
                sB = sp.tile([P, 2, QB], F32, name="sB", tag="s")
                # the deep band k-group only computes scores from the first
                # valid column of its j0 tile; j1's [offs0,offs1) range is
                # real-but-masked so the shared exp/mask APs below never
                # read uninitialized PSUM
                c0 = offs[0] if deep else 0
                group = [
                    _mm(
                        nc,
                        stile[:, j, c0:QB],
                        kt_sb[p][rows, (kg * 2 + j) * P : (kg * 2 + j + 1) * P],
                        qt_sb[p][rows, q0 + c0 : q0 + QB],
                        start=True,
                        stop=True,
                    )
                    for j in range(2)
                    for rows, stile in ((slice(0, 64), sA), (slice(64, P), sB))
                ]
                _chain(group)
                _gate(
                    nc,
                    group[0],
                    [(qksem[(p, "k", kg // 2)], 1), (qksem[(p, "q", qb)], 1)],
                    f"gsc{p}{qb}{kg}",
                )
                pA = ppool.tile([P, 2, QB], MT, name="pA", tag="pt")
                pB = ppool.tile([P, 2, QB], MT, name="pB", tag="pt")
                # exp(S/sqrt(dh)); scale folded into ACT
                if deep:
                    o0 = offs[0]
                    nc.scalar.activation(
                        pA[:, :, o0:], sA[:, :, o0:],
                        mybir.ActivationFunctionType.Exp, scale=0.125,
                    )
                    nc.scalar.activation(
                        pB[:, :, o0:], sB[:, :, o0:],
                        mybir.ActivationFunctionType.Exp, scale=0.125,
                    )
                    masks = [
                        nc.vector.tensor_mul(
                            pA[:, :, o0:], pA[:, :, o0:], bandm_sb[:, 1, :, o0:]
                        ),
                        nc.vector.tensor_mul(
                            pB[:, :, o0:], pB[:, :, o0:], bandm_sb[:, 1, :, o0:]
                        ),
                    ]
                    _sem_nop(nc, masks, msem[(p, qb)], f"m{p}{qb}d")
                else:
                    nc.scalar.activation(
                        pA[:], sA[:], mybir.ActivationFunctionType.Exp,
                        scale=0.125,
                    )
                    nc.scalar.activation(
                        pB[:], sB[:], mybir.ActivationFunctionType.Exp,
                        scale=0.125,
                    )
                    if band:
                        # causal mask: multiply diagonal-band tiles by 0/1
                        masks = [
                            nc.vector.tensor_mul(pA[:], pA[:], bandm_sb[:, 0, :, :]),
                            nc.vector.tensor_mul(pB[:], pB[:], bandm_sb[:, 0, :, :]),
                        ]
                        _sem_nop(nc, masks, msem[(p, qb)], f"m{p}{qb}b")
                return pA, pB

            pts = []
            for kg in range(nkg):
                pts.append(scores_exp(kg))
                if kg > 0:
                    pop_filler(plan[kg - 1])
                    pv_dnb(*pts[kg - 1], kg - 1)
            pop_filler(plan[nkg - 1])
            pv_dnb(*pts[nkg - 1], nkg - 1)

            bcs = bcpool.tile([P, QB], F32, name="bcs", tag="bcs")
            bcr = bcpool.tile([P, QB], F32, name="bcr", tag="bcr")
            nc.vector.reciprocal_approx_accurate(
                out=bcr[:], in_=dnb[:], scratch=bcs[:]
            )
            zi = nc.vector.tensor_mul(zt_sb[p][:, q0 : q0 + QB], zps[:], bcr[:])
            _sem_nop(nc, [zi], zsem[(p, qb)], f"zt{p}{qb}")

        # ---- emission: minimal prelude (first Q/K chains), then the
        # attention blocks ordered so exp starts early and O-projection row
        # groups unlock in time to fill the late blocks ----
        qk_chain(0, 0, "q")
        qk_chain(0, 0, "k")
        attention_qb(0, 0, [2, 2])
        attention_qb(1, 0, [2, 2])
        attention_qb(0, 1, [2, 2, 2, 2])
        attention_qb(1, 1, [2, 2, 2, 2])
        attention_qb(0, 3, [1] * 8)
        attention_qb(1, 3, [1] * 6 + [0] * 2)
        attention_qb(0, 2, [1] * 6)
        attention_qb(1, 2, [1, 1] + [0] * 4)
        # remaining O units (last q block) as the tail
        pop_filler(len(fillers))

    nc.compile()
    _PROGRAM_CACHE[key] = nc
    return nc


def make_in_maps(
    normalized_resid_pre, W_Q, W_K, W_V, W_O, b_Q, b_K, b_V, b_O,
    mm_dtype=MM_DTYPE, with_bias=False,
):
    """Shard + prearrange the full inputs into per-core input maps."""
    np_mt = {"fp16": np.float16, "fp32": np.float32}.get(mm_dtype)
    if np_mt is None:
        import ml_dtypes  # noqa: F401  (registers bfloat16 with numpy)
        np_mt = np.dtype("bfloat16")

    x = np.asarray(normalized_resid_pre, dtype=np.float32)
    W_Q = np.asarray(W_Q, dtype=np.float32)
    W_K = np.asarray(W_K, dtype=np.float32)
    W_V = np.asarray(W_V, dtype=np.float32)
    W_O = np.asarray(W_O, dtype=np.float32)
    b_Q = np.asarray(b_Q, dtype=np.float32)
    b_K = np.asarray(b_K, dtype=np.float32)
    b_V = np.asarray(b_V, dtype=np.float32)

    xT = [np.ascontiguousarray(x[b].T).astype(np_mt) for b in range(B)]
    # multiplicative causal band masks at k-group granularity: variant o
    # covers the two k-tiles at q-block offsets (2o*128, (2o+1)*128)
    kp = np.arange(P)[:, None]
    qc = np.arange(QB)[None, :]
    bandm = np.concatenate(
        [
            np.concatenate(
                [
                    np.where(qc < (2 * o + j) * P + kp,
                             np.float32(0.0), np.float32(1.0))
                    for j in range(2)
                ],
                axis=1,
            )
            for o in range(2)
        ],
        axis=1,
    ).astype(np_mt)

    in_maps = []
    for c in range(NCORES):
        b = c // (NCORES // B)
        heads = [HPC * (c % (NCORES // B)) + i for i in range(HPC)]
        wq = np.concatenate([W_Q[h] for h in heads], axis=1)
        wk = np.concatenate([W_K[h] for h in heads], axis=1)
        wv = np.concatenate([W_V[h] for h in heads], axis=1)
        wqkv = np.concatenate([wq, wk, wv], axis=1).astype(np_mt)
        wo_full = np.concatenate([W_O[h] for h in heads], axis=0)  # [256, 1024]
        wo = np.concatenate([wo_full[0:P], wo_full[P:]], axis=1).astype(np_mt)
        m = {
            "xT": np.ascontiguousarray(xT[b]),
            "wqkv": wqkv,
            "wo": np.ascontiguousarray(wo),
            "bandm": np.ascontiguousarray(bandm),
        }
        if with_bias:
            m["bq"] = np.stack(
                [
                    np.concatenate([b_Q[heads[0]], b_Q[heads[1]]]),
                    np.concatenate([b_Q[heads[2]], b_Q[heads[3]]]),
                ]
            ).astype(np.float32)
            m["bk"] = np.stack(
                [
                    np.concatenate([b_K[heads[0]], b_K[heads[1]]]),
                    np.concatenate([b_K[heads[2]], b_K[heads[3]]]),
                ]
            ).astype(np.float32)
            m["bv"] = np.tile(
                np.concatenate([b_V[h] for h in heads])[None, :], (P, 1)
            ).astype(np.float32)
        in_maps.append(m)
    return in_maps


def kernel(normalized_resid_pre, W_Q, W_K, W_V, W_O, b_Q, b_K, b_V, b_O):
    global LAST_RESULTS
    with_bias = any(
        np.any(np.asarray(bx)) for bx in (b_Q, b_K, b_V)
    )
    nc = build_program(MM_DTYPE, with_bias)
    in_maps = make_in_maps(
        normalized_resid_pre, W_Q, W_K, W_V, W_O, b_Q, b_K, b_V, b_O,
        MM_DTYPE, with_bias,
    )
    trace = os.environ.get("ATTN_TRACE", "0") == "1"
    res = run_bass_kernel_spmd(nc, in_maps, list(range(NCORES)), trace=trace)
    LAST_RESULTS = res

    b_O = np.asarray(b_O, dtype=np.float32)
    # per-core partials are O^T [d_model, seq]; transpose while summing
    parts = [np.asarray(res.results[c]["out"], dtype=np.float32) for c in range(NCORES)]
    npc = NCORES // B  # cores per batch
    out = np.stack(
        [sum(parts[b * npc : (b + 1) * npc]).T + b_O for b in range(B)]
    )
    return out.astype(np.float32)


# revision 69
# speedup vs baseline: 1.0649x; 1.0174x over previous
"""Trainium2 Bass kernel for causal multi-head attention (dense transformer).

Problem (hardcoded): x [2, 2048, 1024], 16 heads x 64 dh, causal attention,
fp32 I/O. Sharding: 8 cores = 2 batches x 4 head-groups. Each core computes 4
heads for one batch plus a partial output projection [2048, 1024]; the host
sums the 4 partials per batch and adds b_O.

Everything on-device is computed in "transposed" orientation so no transposes
are needed anywhere:
  x^T (host-pretransposed)  ->  Q^T, K^T [dh, s] and V [s, dh] via matmuls
  S^T[k, q] = K Q^T         ->  P^T = exp(S^T / 8) (causal-masked)
  Z^T[dh, q] = V^T P^T      ->  normalized by column sums (ones-matmul)
  O^T[d, q] = W_O^T Z^T     (zt as the moving operand; host transposes)

Heads are processed in pairs: QK^T packs 2 heads in row-groups (0-63 / 64-127)
of the PE array, PV packs 2 heads in column-groups -- both run concurrently.

Schedule: attention q-blocks are software-pipelined (next scores emitted
before previous PV so the exp stream feeds the ACT engine early) and
interleaved with "filler" PE work (remaining QKV projection chains, then
O-projection chunks) so the PE stays busy while ACT churns exp. Input DMAs
are column-sliced and spread across idle engine queues. All on-device
compute is fp16; output partials are DMA'd as fp16 and summed on host.

Correctness note: this Tile build drops cross-engine waits for consumers of
DVE-written persist tiles (qt/kt/v/zt casts) -- first-run races showed up as
NaNs. Every such edge is therefore gated by a raw semaphore: the producing
DVE op is followed by a same-queue nop carrying then_inc, and consuming
matmuls/DMAs carry explicit wait_ops (see _sem_nop/_gate).
"""

import os
from contextlib import ExitStack

import numpy as np

import concourse.tile as tile
from concourse import bacc, mybir
from concourse.bass_utils import run_bass_kernel_spmd

# problem constants
B, S, DM, H, DH = 2, 2048, 1024, 16, 64
P = 128          # partitions
QB = 512         # q block (matmul moving free dim)
NKT = S // P     # 16 k tiles
NQB = S // QB    # 4 q blocks
NDM = DM // P    # 8 d_model tiles
HPC = 4          # heads per core
NCORES = 8

F32 = mybir.dt.float32
F16 = mybir.dt.float16
BF16 = mybir.dt.bfloat16

MM_DTYPE = os.environ.get("ATTN_MM_DTYPE", "fp16")

_PROGRAM_CACHE = {}
LAST_RESULTS = None  # BassKernelResults of the most recent run (for test.py)


def _mm(nc, out, lhsT, rhs, start, stop, skip=False):
    # skip_group_check: the sim's psum-group tracker doesn't distinguish
    # partition ranges; our concurrent groups in one bank are partition-disjoint
    # (rows 0-63 vs 64-127), which the per-partition zeroing model handles.
    return nc.tensor.matmul(
        out, lhsT, rhs, start=start, stop=stop, skip_group_check=skip
    )


def _chain(insts):
    """Same-engine ordering edges: pins the group's static queue order so a
    wait on the FIRST matmul gates the whole group (row/column-packed pairs
    still run concurrently on the array -- order only fixes dispatch)."""
    from concourse.tile import add_dep_helper

    for a, b in zip(insts[1:], insts):
        add_dep_helper(a.ins, b.ins, sync=True, reason="pack-pair order")


def _gate(nc, first_mm, waits, hint):
    """Gate a (chained) matmul group on raw semaphores. An instruction holds
    one raw wait; extra waits ride on tensor-queue nops ordered before the
    group's first matmul."""
    from concourse.tile import add_dep_helper

    sem, val = waits[0]
    first_mm.wait_op(sem, val, "sem-ge")
    for i, (sem, val) in enumerate(waits[1:]):
        tn = nc.tensor.nop(nofuse=True, hint=f"{hint}{i}")
        tn.wait_op(sem, val, "sem-ge")
        add_dep_helper(first_mm.ins, tn.ins, sync=True, reason="gate order")


def _sem_nop(nc, producers, sem, hint):
    """Vector-queue nop ordered after `producers` (same-engine sync edges)
    carrying a raw semaphore increment. This build's Tile scheduler elides
    cross-engine waits it believes are timing-covered -- unsoundly when the
    sim underestimates DVE/DMA latency -- so consumers of DVE-produced data
    wait on these raw semaphores instead."""
    from concourse.tile import add_dep_helper

    nop = nc.vector.nop(nofuse=True, hint=hint)
    for pr in producers:
        add_dep_helper(nop.ins, pr.ins, sync=True, reason="sem-nop order")
    nop.then_inc(sem)
    return nop


def build_program(mm_dtype=MM_DTYPE, with_bias=False):
    """Build the single-core SPMD Bass program (same program on all 8 cores)."""
    key = (mm_dtype, with_bias)
    if key in _PROGRAM_CACHE:
        return _PROGRAM_CACHE[key]

    MT = {"fp16": F16, "bf16": BF16, "fp32": F32}[mm_dtype]

    nc = bacc.Bacc(
        "TRN2", target_bir_lowering=False, debug=False, num_devices=NCORES
    )

    # ---- DRAM I/O (per-core shards, prearranged on host) ----
    xT_d = nc.dram_tensor("xT", [DM, S], MT, kind="ExternalInput")
    wqkv_d = nc.dram_tensor("wqkv", [DM, 3 * HPC * DH], MT, kind="ExternalInput")
    wo_d = nc.dram_tensor("wo", [P, 2 * DM], MT, kind="ExternalInput")
    bandm_d = nc.dram_tensor("bandm", [P, 2 * 2 * QB], MT, kind="ExternalInput")
    if with_bias:
        bq_d = nc.dram_tensor("bq", [2, P], F32, kind="ExternalInput")
        bk_d = nc.dram_tensor("bk", [2, P], F32, kind="ExternalInput")
        bv_d = nc.dram_tensor("bv", [P, HPC * DH], F32, kind="ExternalInput")
    # output is O^T [d_model, seq] so zt is the matmul's MOVING operand (its
    # cross-engine dependency is enforced; a zt lhsT read raced) -- the host
    # transposes while summing partials
    out_d = nc.dram_tensor("out", [DM, S], F16, kind="ExternalOutput")

    with tile.TileContext(nc) as tc, ExitStack() as ctx:
        const = ctx.enter_context(tc.tile_pool(name="const", bufs=1))
        persist = ctx.enter_context(tc.tile_pool(name="persist", bufs=1))

        # ---- persistent SBUF ----
        xt_sb = persist.tile([P, NDM, S], MT, name="xt_sb", tag="xt")
        w_sb = persist.tile([P, NDM, 3 * HPC * DH], MT, name="w_sb", tag="w")
        wo_sb = persist.tile([P, 2, DM], MT, name="wo_sb", tag="wo")
        bandm_sb = const.tile([P, 2, 2, QB], MT, name="bandm_sb", tag="bandm")
        ones64 = const.tile([P, 64], MT, name="ones64", tag="ones64")
        qt_sb = [persist.tile([P, S], MT, name=f"qt{p}", tag=f"qt{p}") for p in range(2)]
        kt_sb = [persist.tile([P, S], MT, name=f"kt{p}", tag=f"kt{p}") for p in range(2)]
        v_sb = [persist.tile([P, NKT, P], MT, name=f"v{p}", tag=f"v{p}") for p in range(2)]
        zt_sb = [persist.tile([P, S], MT, name=f"zt{p}", tag=f"zt{p}") for p in range(2)]

        if with_bias:
            bq_sb = const.tile([P, 2], F32, name="bq_sb", tag="bq")
            bk_sb = const.tile([P, 2], F32, name="bk_sb", tag="bk")
            bv_sb = const.tile([P, HPC * DH], F32, name="bv_sb", tag="bv")

        # ---- input DMAs: column-sliced x for an early start, spread across
        # otherwise-idle engine queues (issue cost ~0.6us per 128-row DMA) ----
        nc.gpsimd.memset(ones64[:], 1.0)
        # first-needed tiles (weights + x columns 0:512) split across three
        # queues so no single stream paces the opening projection chains
        for t in range(NDM):
            weng = nc.sync if t % 2 == 0 else nc.scalar
            weng.dma_start(out=w_sb[:, t, :], in_=wqkv_d[t * P : (t + 1) * P, :])
            xeng = nc.gpsimd if t % 2 == 0 else nc.sync
            xeng.dma_start(
                out=xt_sb[:, t, 0:QB], in_=xT_d[t * P : (t + 1) * P, 0:QB]
            )
        for o in range(2):
            nc.sync.dma_start(
                out=bandm_sb[:, o, :, :],
                in_=bandm_d[:, o * 2 * QB : (o + 1) * 2 * QB],
            )
        for t in range(NDM):
            nc.scalar.dma_start(
                out=xt_sb[:, t, QB : 2 * QB],
                in_=xT_d[t * P : (t + 1) * P, QB : 2 * QB],
            )
        nc.scalar.dma_start(out=wo_sb[:, :, :], in_=wo_d[:, :])
        for t in range(NDM):
            nc.sync.dma_start(
                out=xt_sb[:, t, 2 * QB :], in_=xT_d[t * P : (t + 1) * P, 2 * QB :]
            )
        if with_bias:
            for p in range(2):
                nc.sync.dma_start(out=bq_sb[:, p : p + 1], in_=bq_d[p : p + 1, :])
                nc.sync.dma_start(out=bk_sb[:, p : p + 1], in_=bk_d[p : p + 1, :])
            nc.sync.dma_start(out=bv_sb[:], in_=bv_d[:, :])

        # ---- psum pools: sp shared by scores / projections / O-chunks ----
        sp = ctx.enter_context(tc.tile_pool(name="sp", bufs=3, space="PSUM"))
        zp = ctx.enter_context(tc.tile_pool(name="zp", bufs=1, space="PSUM"))
        dp = ctx.enter_context(tc.tile_pool(name="dp", bufs=1, space="PSUM"))
        ppool = ctx.enter_context(tc.tile_pool(name="ppool", bufs=16))
        bcpool = ctx.enter_context(tc.tile_pool(name="bcpool", bufs=2))
        ost = ctx.enter_context(tc.tile_pool(name="ost", bufs=6))

        # raw semaphores forcing every DVE -> PE/DMA cross-engine sync that
        # Tile's scheduler has been observed to drop (first-run NaNs/garbage
        # from matmuls reading casts or zt before the producing DVE op ran)
        zsem = {
            (p, qb): nc.alloc_semaphore(name=f"zsem{p}_{qb}")
            for p in range(2)
            for qb in range(NQB)
        }
        qksem = {
            (p, w, ch): nc.alloc_semaphore(name=f"qksem{p}{w}{ch}")
            for p in range(2)
            for w in ("q", "k")
            for ch in range(NQB)
        }
        vsem = {
            (st): nc.alloc_semaphore(name=f"vsem{st}")
            for st in range(0, NKT, 2)
        }
        msem = {
            (p, qb): nc.alloc_semaphore(name=f"msem{p}_{qb}")
            for p in range(2)
            for qb in range(NQB)
        }
        osem = {
            (t, qb): nc.alloc_semaphore(name=f"osem{t}_{qb}")
            for t in range(NDM)
            for qb in range(NQB)
        }

        def qk_chain(p, ch, which):
            """One Q^T or K^T projection chain: [dh-pair(128), 512 q cols]."""
            base, dst = (0, qt_sb) if which == "q" else (2 * P, kt_sb)
            bias = None
            if with_bias:
                bias = bq_sb if which == "q" else bk_sb
            qp = sp.tile([P, 2, QB], F32, name="qp", tag="s")
            for t in range(NDM):
                _mm(
                    nc,
                    qp[:, 0, :],
                    w_sb[:, t, base + p * P : base + (p + 1) * P],
                    xt_sb[:, t, ch * QB : (ch + 1) * QB],
                    start=(t == 0),
                    stop=(t == NDM - 1),
                )
            out = dst[p][:, ch * QB : (ch + 1) * QB]
            if with_bias:
                inst = nc.vector.tensor_scalar_add(
                    out, qp[:, 0, :], bias[:, p : p + 1]
                )
            else:
                inst = nc.vector.tensor_copy(out, qp[:, 0, :])
            _sem_nop(nc, [inst], qksem[(p, which, ch)], f"qk{p}{which}{ch}")

        def v_chain2(st):
            """V rows [st*128, (st+2)*128) for all 4 heads (two 8-MM chains
            into one psum tile, one strided cast per head pair)."""
            vp = sp.tile([P, 2, QB], F32, name="vp", tag="s")
            for half in range(2):
                for t in range(NDM):
                    _mm(
                        nc,
                        vp[:, half, 0 : HPC * DH],
                        xt_sb[:, t, (st + half) * P : (st + half + 1) * P],
                        w_sb[:, t, 4 * P : 4 * P + HPC * DH],
                        start=(t == 0),
                        stop=(t == NDM - 1),
                    )
            casts = []
            for p in range(2):
                if with_bias:
                    for half in range(2):
                        casts.append(nc.vector.tensor_add(
                            v_sb[p][:, st + half, :],
                            vp[:, half, p * P : (p + 1) * P],
                            bv_sb[:, p * P : (p + 1) * P],
                        ))
                else:
                    casts.append(nc.vector.tensor_copy(
                        v_sb[p][:, st : st + 2, :], vp[:, :, p * P : (p + 1) * P]
                    ))
            _sem_nop(nc, casts, vsem[st], f"v{st}")

        def o_unit(t, qb):
            """Partial O^T chunk [t*128,(t+1)*128) x [qb*512,(qb+1)*512):
            two accumulating MMs (one per head pair) with zt as the moving
            operand, one cast, one DMA."""
            ops = sp.tile([P, 2, QB], F32, name="ops", tag="s")
            mms = []
            for pp in range(2):
                mms.append(_mm(
                    nc,
                    ops[:, 0, :],
                    wo_sb[:, pp, t * P : (t + 1) * P],
                    zt_sb[pp][:, qb * QB : (qb + 1) * QB],
                    start=(pp == 0),
                    stop=(pp == 1),
                ))
                mms[pp].wait_op(zsem[(pp, qb)], 1, "sem-ge")
            _chain(mms)
            ot = ost.tile([P, QB], F16, name="ot", tag="ot")
            ocast = nc.vector.tensor_copy(ot[:, :], ops[:, 0, :])
            # per-chunk sem so the output DMA reads ot only after the cast;
            # alternate issue queues so the tail's issues don't serialize
            _sem_nop(nc, [ocast], osem[(t, qb)], f"o{t}_{qb}")
            deng = nc.gpsimd if (t + qb) % 2 == 0 else nc.sync
            deng.dma_start(
                out=out_d[t * P : (t + 1) * P, qb * QB : (qb + 1) * QB],
                in_=ot[:, :],
            ).wait_op(osem[(t, qb)], 1, "sem-ge")

        # ---- filler iterator: independent PE work interleaved into the
        # attention kg-step stream to keep the PE busy while ACT does exp ----
        fillers = [
            ("v", 0), ("v", 2), ("qk", 1, 0, "q"), ("qk", 1, 0, "k"),        # b1
            ("qk", 0, 1, "q"), ("qk", 0, 1, "k"), ("v", 4), ("v", 6),        # b2
            ("qk", 1, 1, "q"), ("qk", 1, 1, "k"),
            ("qk", 0, 2, "q"), ("qk", 0, 2, "k"),
            ("v", 8), ("v", 10), ("qk", 0, 3, "q"), ("qk", 0, 3, "k"),       # b3
            ("v", 12), ("v", 14), ("qk", 1, 3, "q"), ("qk", 1, 3, "k"),
            ("qk", 1, 2, "q"), ("qk", 1, 2, "k"), ("o", 0, 0), ("o", 1, 0),  # b4
        ] + [("o", t, 0) for t in range(2, NDM)] + [("o", 0, 1), ("o", 1, 1)] \
          + [("o", t, 1) for t in range(2, NDM)] \
          + [("o", t, 3) for t in range(NDM)] \
          + [("o", t, 2) for t in range(NDM)]
        fill_i = [0]

        def pop_filler(n):
            for _ in range(n):
                if fill_i[0] >= len(fillers):
                    return
                f = fillers[fill_i[0]]
                fill_i[0] += 1
                if f[0] == "qk":
                    qk_chain(f[1], f[2], f[3])
                elif f[0] == "v":
                    v_chain2(f[1])
                else:
                    o_unit(f[1], f[2])

        def attention_qb(p, qb, plan):
            """One attention q-block, software-pipelined: scores/exp of step
            s are emitted before PV of step s-1 so the PE prioritizes feeding
            the ACT engine; plan[s] fillers are popped after each PV."""
            q0 = qb * QB
            nk = (qb + 1) * (QB // P)  # k tiles in causal range
            nkg = nk // 2
            zps = zp.tile([P, QB], F32, name="zps", tag="z")
            dnb = dp.tile([P, QB], F32, name="dnb", tag="d")

            def pv_dnb(pA, pB, kg):
                # PV (column-packed heads) + softmax denominators: the
                # ones-matmul sums P over k AND broadcasts over the 64
                # rows of each head half, accumulated in PSUM; all read
                # only the valid q range of their k-tile
                group = []
                for j in range(2):
                    kt = kg * 2 + j
                    c0 = max(kt * P - q0, 0)
                    group += [
                        _mm(
                            nc, zps[0:64, c0:QB], v_sb[p][:, kt, 0:64],
                            pA[:, j, c0:QB],
                            start=(kt == 0), stop=(kt == nk - 1), skip=True,
                        ),
                        _mm(
                            nc, zps[64:P, c0:QB], v_sb[p][:, kt, 64:P],
                            pB[:, j, c0:QB],
                            start=(kt == 0), stop=(kt == nk - 1), skip=True,
                        ),
                        _mm(
                            nc, dnb[0:64, c0:QB], ones64[:], pA[:, j, c0:QB],
                            start=(kt == 0), stop=(kt == nk - 1), skip=True,
                        ),
                        _mm(
                            nc, dnb[64:P, c0:QB], ones64[:], pB[:, j, c0:QB],
                            start=(kt == 0), stop=(kt == nk - 1), skip=True,
                        ),
                    ]
                _chain(group)
                waits = [(vsem[kg * 2], 1)]
                if kg >= 2 * qb:
                    # this k-group's P tiles were rewritten by the band mask
                    waits.append((msem[(p, qb)], kg - 2 * qb + 1))
                _gate(nc, group[0], waits, f"gpv{p}{qb}{kg}")

            def scores_exp(kg):
                # offs[j]: first valid q column of k-tile kg*2+j
                offs = [kg * 2 * P + j * P - q0 for j in range(2)]
                band = offs[0] >= 0
                deep = band and offs[0] >= 2 * P  # o=1 band k-group
                sA = sp.tile([P, 2, QB], F32, name="sA", tag="s")
                sB = sp.tile([P, 2, QB], F32, name="sB", tag="s")
                # the deep band k-group only computes scores from the first
                # valid column of its j0 tile; j1's [offs0,offs1) range is
                # real-but-masked so the shared exp/mask APs below never
                # read uninitialized PSUM
                c0 = offs[0] if deep else 0
                group = [
                    _mm(
                        nc,
                        stile[:, j, c0:QB],
                        kt_sb[p][rows, (kg * 2 + j) * P : (kg * 2 + j + 1) * P],
                        qt_sb[p][rows, q0 + c0 : q0 + QB],
                        start=True,
                        stop=True,
                    )
                    for j in range(2)
                    for rows, stile in ((slice(0, 64), sA), (slice(64, P), sB))
                ]
                _chain(group)
                _gate(
                    nc,
                    group[0],
                    [(qksem[(p, "k", kg // 2)], 1), (qksem[(p, "q", qb)], 1)],
                    f"gsc{p}{qb}{kg}",
                )
                pA = ppool.tile([P, 2, QB], MT, name="pA", tag="pt")
                pB = ppool.tile([P, 2, QB], MT, name="pB", tag="pt")
                # exp(S/sqrt(dh)); scale folded into ACT
                if deep:
                    o0 = offs[0]
                    nc.scalar.activation(
                        pA[:, :, o0:], sA[:, :, o0:],
                        mybir.ActivationFunctionType.Exp, scale=0.125,
                    )
                    nc.scalar.activation(
                        pB[:, :, o0:], sB[:, :, o0:],
                        mybir.ActivationFunctionType.Exp, scale=0.125,
                    )
                    masks = [
                        nc.vector.tensor_mul(
                            pA[:, :, o0:], pA[:, :, o0:], bandm_sb[:, 1, :, o0:]
                        ),
                        nc.vector.tensor_mul(
                            pB[:, :, o0:], pB[:, :, o0:], bandm_sb[:, 1, :, o0:]
                        ),
                    ]
                    _sem_nop(nc, masks, msem[(p, qb)], f"m{p}{qb}d")
                else:
                    nc.scalar.activation(
                        pA[:], sA[:], mybir.ActivationFunctionType.Exp,
                        scale=0.125,
                    )
                    nc.scalar.activation(
                        pB[:], sB[:], mybir.ActivationFunctionType.Exp,
                        scale=0.125,
                    )
                    if band:
                        # causal mask: multiply diagonal-band tiles by 0/1
                        masks = [
                            nc.vector.tensor_mul(pA[:], pA[:], bandm_sb[:, 0, :, :]),
                            nc.vector.tensor_mul(pB[:], pB[:], bandm_sb[:, 0, :, :]),
                        ]
                        _sem_nop(nc, masks, msem[(p, qb)], f"m{p}{qb}b")
                return pA, pB

            pts = []
            for kg in range(nkg):
                pts.append(scores_exp(kg))
                if kg > 0:
                    pop_filler(plan[kg - 1])
                    pv_dnb(*pts[kg - 1], kg - 1)
            pop_filler(plan[nkg - 1])
            pv_dnb(*pts[nkg - 1], nkg - 1)

            bcr = bcpool.tile([P, QB], F32, name="bcr", tag="bcr")
            # fast 1-op reciprocal (~18 correct bits, far inside the fp16
            # error budget) halves the qb-boundary critical-path latency
            nc.vector.reciprocal_approx_fast(out=bcr[:], in_=dnb[:])
            zi = nc.vector.tensor_mul(zt_sb[p][:, q0 : q0 + QB], zps[:], bcr[:])
            _sem_nop(nc, [zi], zsem[(p, qb)], f"zt{p}{qb}")

        # ---- emission: minimal prelude (first Q/K chains), then the
        # attention blocks ordered so exp starts early and O-projection row
        # groups unlock in time to fill the late blocks ----
        qk_chain(0, 0, "q")
        qk_chain(0, 0, "k")
        attention_qb(0, 0, [2, 2])
        attention_qb(1, 0, [2, 2])
        attention_qb(0, 1, [2, 2, 2, 2])
        attention_qb(1, 1, [2, 2, 2, 2])
        attention_qb(0, 3, [1] * 8)
        attention_qb(1, 3, [1] * 6 + [0] * 2)
        attention_qb(0, 2, [1] * 6)
        attention_qb(1, 2, [1, 1] + [0] * 4)
        # remaining O units (last q block) as the tail
        pop_filler(len(fillers))

    nc.compile()
    _PROGRAM_CACHE[key] = nc
    return nc


def make_in_maps(
    normalized_resid_pre, W_Q, W_K, W_V, W_O, b_Q, b_K, b_V, b_O,
    mm_dtype=MM_DTYPE, with_bias=False,
):
    """Shard + prearrange the full inputs into per-core input maps."""
    np_mt = {"fp16": np.float16, "fp32": np.float32}.get(mm_dtype)
    if np_mt is None:
        import ml_dtypes  # noqa: F401  (registers bfloat16 with numpy)
        np_mt = np.dtype("bfloat16")

    x = np.asarray(normalized_resid_pre, dtype=np.float32)
    W_Q = np.asarray(W_Q, dtype=np.float32)
    W_K = np.asarray(W_K, dtype=np.float32)
    W_V = np.asarray(W_V, dtype=np.float32)
    W_O = np.asarray(W_O, dtype=np.float32)
    b_Q = np.asarray(b_Q, dtype=np.float32)
    b_K = np.asarray(b_K, dtype=np.float32)
    b_V = np.asarray(b_V, dtype=np.float32)

    xT = [np.ascontiguousarray(x[b].T).astype(np_mt) for b in range(B)]
    # multiplicative causal band masks at k-group granularity: variant o
    # covers the two k-tiles at q-block offsets (2o*128, (2o+1)*128)
    kp = np.arange(P)[:, None]
    qc = np.arange(QB)[None, :]
    bandm = np.concatenate(
        [
            np.concatenate(
                [
                    np.where(qc < (2 * o + j) * P + kp,
                             np.float32(0.0), np.float32(1.0))
                    for j in range(2)
                ],
                axis=1,
            )
            for o in range(2)
        ],
        axis=1,
    ).astype(np_mt)

    in_maps = []
    for c in range(NCORES):
        b = c // (NCORES // B)
        heads = [HPC * (c % (NCORES // B)) + i for i in range(HPC)]
        wq = np.concatenate([W_Q[h] for h in heads], axis=1)
        wk = np.concatenate([W_K[h] for h in heads], axis=1)
        wv = np.concatenate([W_V[h] for h in heads], axis=1)
        wqkv = np.concatenate([wq, wk, wv], axis=1).astype(np_mt)
        wo_full = np.concatenate([W_O[h] for h in heads], axis=0)  # [256, 1024]
        wo = np.concatenate([wo_full[0:P], wo_full[P:]], axis=1).astype(np_mt)
        m = {
            "xT": np.ascontiguousarray(xT[b]),
            "wqkv": wqkv,
            "wo": np.ascontiguousarray(wo),
            "bandm": np.ascontiguousarray(bandm),
        }
        if with_bias:
            m["bq"] = np.stack(
                [
                    np.concatenate([b_Q[heads[0]], b_Q[heads[1]]]),
                    np.concatenate([b_Q[heads[2]], b_Q[heads[3]]]),
                ]
            ).astype(np.float32)
            m["bk"] = np.stack(
                [
                    np.concatenate([b_K[heads[0]], b_K[heads[1]]]),
                    np.concatenate([b_K[heads[2]], b_K[heads[3]]]),
                ]
            ).astype(np.float32)
            m["bv"] = np.tile(
                np.concatenate([b_V[h] for h in heads])[None, :], (P, 1)
            ).astype(np.float32)
        in_maps.append(m)
    return in_maps


def kernel(normalized_resid_pre, W_Q, W_K, W_V, W_O, b_Q, b_K, b_V, b_O):
    global LAST_RESULTS
    with_bias = any(
        np.any(np.asarray(bx)) for bx in (b_Q, b_K, b_V)
    )
    nc = build_program(MM_DTYPE, with_bias)
    in_maps = make_in_maps(
        normalized_resid_pre, W_Q, W_K, W_V, W_O, b_Q, b_K, b_V, b_O,
        MM_DTYPE, with_bias,
    )
    trace = os.environ.get("ATTN_TRACE", "0") == "1"
    res = run_bass_kernel_spmd(nc, in_maps, list(range(NCORES)), trace=trace)
    LAST_RESULTS = res

    b_O = np.asarray(b_O, dtype=np.float32)
    # per-core partials are O^T [d_model, seq]; transpose while summing
    parts = [np.asarray(res.results[c]["out"], dtype=np.float32) for c in range(NCORES)]
    npc = NCORES // B  # cores per batch
    out = np.stack(
        [sum(parts[b * npc : (b + 1) * npc]).T + b_O for b in range(B)]
    )
    return out.astype(np.float32)
